# revision 1
# baseline (speedup 1.0000x reference)
"""Trainium2 Bass kernel for nn_DoubleNet (two GATNet branches + avg-pool + linear).

Strategy (8 NeuronCores):
  - Cores 0-3 run branch A, cores 4-7 run branch B (same SPMD program,
    different input data per core).
  - Within a branch, dst nodes are sharded contiguously across the 4 cores.
  - Per GAT layer:
      dense phase: every core computes the full z_aug = x @ [W | W@al | W@ar]
        table (z interleaved with ones-columns for the softmax denominator,
        el/er appended in the row tail) and writes it to its own DRAM; the
        er-side gather reads the 256B-aligned row tail of the same table.
      edge phase: edges are pre-sorted by dst (host side) and processed in
        chunks of 128; per chunk a dma_gather pulls z_aug[src] rows, attention
        weights w = exp(leaky_relu(el[src]+er[dst])) are computed per edge, and
        a w-scaled one-hot matmul on the PE scatter-adds messages AND the
        denominator into PSUM per 128-dst block.
      linear: x_next^T = Wl^T @ agg^T computed per block; shards are
        all-gathered across the 4 cores of the branch.
  - Final layer pools via a host-precomputed gid one-hot matmul; host sums the
    per-core partial pools and applies the output linear.
"""

import sys

sys.path.insert(0, "/opt/trn_rl_repo")

import numpy as np


# ---------------------------------------------------------------------------
# configuration
# ---------------------------------------------------------------------------

class Cfg:
    def __init__(self, N=20000, G=128, H=3, EMB=128, F=128, n_cores=8,
                 table_bf16=True, neg_slope=0.2):
        assert F == 128 and EMB == 128 and H == 3
        self.N, self.G, self.H, self.EMB, self.F = N, G, H, EMB, F
        self.n_cores = n_cores
        self.gpb = n_cores // 2            # cores per branch
        assert N % self.gpb == 0
        self.SH = N // self.gpb            # dst nodes per core
        self.NB = -(-self.SH // 128)       # dst blocks per core
        self.NT = -(-N // 128)             # node chunks for dense phase
        self.NTP = self.NT * 128           # padded node count
        self.neg_slope = neg_slope
        self.table_bf16 = table_bf16
        if table_bf16:
            self.ZC, self.EC = 512, 128    # table cols (bf16: 1024B / 256B)
        else:
            self.ZC, self.EC = 448, 64     # f32: 1792B / 256B rows
        self.GC = 8                        # chunks per z-gather call
        self.dma_scratch = 16384           # SWDGE ring carveout bytes/partition
        # z_aug column layout: z0|1|z1|1|z2|1|el  -> used cols = 390
        self.EL0 = 387                     # el columns 387:390


# ---------------------------------------------------------------------------
# host-side data prep
# ---------------------------------------------------------------------------

def _prep_edges(cfg, src, dst, q):
    """Edges of one core (dst in its shard), dst-sorted, fake rows added."""
    lo = q * cfg.SH
    sel = (dst >= lo) & (dst < lo + cfg.SH)
    es = src[sel].astype(np.int64)
    ed = (dst[sel].astype(np.int64) - lo)
    nfake = cfg.NB * 128 - cfg.SH
    if nfake:
        es = np.concatenate([es, np.zeros(nfake, np.int64)])
        ed = np.concatenate([ed, np.arange(cfg.SH, cfg.NB * 128, dtype=np.int64)])
    order = np.argsort(ed, kind="stable")
    es, ed = es[order], ed[order]
    cnt = np.bincount(ed // 128, minlength=cfg.NB)
    return es, ed, cnt, lo


def _pack_core(cfg, es, ed, lo, nc_b):
    """Build flat (block, chunk, slot) arrays padded to nc_b chunks/block."""
    TOT = int(nc_b.sum())
    zsrc = np.zeros(TOT * 128, np.int16)
    edst = np.zeros(TOT * 128, np.int16)
    dst3 = np.full(TOT * 128, -1.0, np.float32)
    epos = np.searchsorted(ed, np.arange(0, cfg.NB * 128 + 1, 128))
    c0 = 0
    for b in range(cfg.NB):
        s, e = epos[b], epos[b + 1]
        n = e - s
        o = c0 * 128
        zsrc[o:o + n] = es[s:e]
        # fake rows (local id >= SH) must not use an out-of-range er index
        ei = ed[s:e] + lo
        ei[ed[s:e] >= cfg.SH] = 0
        edst[o:o + n] = ei
        dst3[o:o + n] = (ed[s:e] - b * 128).astype(np.float32)
        c0 += nc_b[b]
    # index tiles: flat i -> (partition i%16, col i//16), replicated to 128 rows
    def wrap(a):
        return np.tile(a.reshape(-1, 16).T, (8, 1)).copy()
    # dst3 partition-major: [128, TOT]
    d3 = dst3.reshape(TOT, 128).T.copy()
    return wrap(zsrc), wrap(edst), d3


def _prep_branch_weights(cfg, W1, al1, ar1, b1, Wn, aln, arn, bn, Wl, bl):
    H, EMB = cfg.H, cfg.EMB

    def waug(W, al, ar):
        K = W.shape[0]
        out = np.zeros((K, 390), np.float32)
        out[:, :384] = W
        for h in range(H):
            out[:, 384 + h] = W[:, h * EMB:(h + 1) * EMB] @ al[h]
            out[:, 387 + h] = W[:, h * EMB:(h + 1) * EMB] @ ar[h]
        return out

    wl3 = Wl.reshape(3, 128, EMB).astype(np.float32)
    blp1 = (b1 @ Wl + bl).astype(np.float32)
    blpn = (bn @ Wl + bl).astype(np.float32)
    return waug(W1, al1, ar1), waug(Wn, aln, arn), wl3, blp1, blpn


# ---------------------------------------------------------------------------
# device program
# ---------------------------------------------------------------------------

def build_program(cfg, nc_b, timing_mode=False, skip=()):
    import concourse.bass as bass
    import concourse.mybir as mybir
    import concourse.tile as tile
    from concourse import bacc

    dt = mybir.dt
    f32 = dt.float32
    DTT = dt.bfloat16 if cfg.table_bf16 else dt.float32
    Alu = mybir.AluOpType
    Act = mybir.ActivationFunctionType

    NB, NT, SH, GC = cfg.NB, cfg.NT, cfg.SH, cfg.GC
    ZC, EC, EL0 = cfg.ZC, cfg.EC, cfg.EL0
    TOT = int(nc_b.sum())
    cum = np.concatenate([[0], np.cumsum(nc_b)]).astype(int)
    gpb = cfg.gpb
    groups = [list(range(gpb)), list(range(gpb, 2 * gpb))]

    nc = bacc.Bacc("TRN2", target_bir_lowering=False, debug=False,
                   num_devices=cfg.n_cores,
                   dynamic_dma_scratch_size=cfg.dma_scratch)

    # inputs -----------------------------------------------------------------
    xT0 = nc.dram_tensor("xT0", [128, cfg.NTP], f32, kind="ExternalInput")
    waug1_d = nc.dram_tensor("waug1", [128, 390], f32, kind="ExternalInput")
    waugn_d = nc.dram_tensor("waugn", [128, 390], f32, kind="ExternalInput")
    wl3_d = nc.dram_tensor("wl3", [3, 128, 128], f32, kind="ExternalInput")
    blp1_d = nc.dram_tensor("blp1", [128, 1], f32, kind="ExternalInput")
    blpn_d = nc.dram_tensor("blpn", [128, 1], f32, kind="ExternalInput")
    iota_d = nc.dram_tensor("iota", [128, 128], DTT, kind="ExternalInput")
    ident_d = nc.dram_tensor("ident", [128, 128], f32, kind="ExternalInput")
    dst3_d = nc.dram_tensor("dst3", [128, TOT], f32, kind="ExternalInput")
    zidx_d = nc.dram_tensor("zidx", [128, TOT * 8], dt.int16, kind="ExternalInput")
    eidx_d = nc.dram_tensor("eidx", [128, TOT * 8], dt.int16, kind="ExternalInput")
    poolw_d = nc.dram_tensor("poolw", [NB, 128, 128], f32, kind="ExternalInput")
    pool_out = nc.dram_tensor("pool_out", [128, 128], f32, kind="ExternalOutput")

    # internal DRAM ----------------------------------------------------------
    zaug = nc.dram_tensor("zaug", [cfg.NTP, ZC], DTT)
    HB = NB // 2
    SH1 = min(HB * 128, SH)
    SH2 = SH - SH1
    xsh1 = nc.dram_tensor("xsh1", [128, SH1], f32)
    xsh2 = nc.dram_tensor("xsh2", [128, SH2], f32)
    xgath1 = nc.dram_tensor("xgath1", [gpb, 128, SH1], f32)
    xgath2 = nc.dram_tensor("xgath2", [gpb, 128, SH2], f32)

    def do_gather(nc, which, timing_mode):
        xs, xg = (xsh1, xgath1) if which == 0 else (xsh2, xgath2)
        if timing_mode:
            for j in range(gpb):
                nc.sync.dma_start(xg.ap()[j], xs.ap())
        else:
            nc.gpsimd.collective_compute(
                "AllGather", mybir.AluOpType.bypass, replica_groups=groups,
                ins=[xs.ap()], outs=[xg.ap()])

    with tile.TileContext(nc) as tc:
        cpool = tc.alloc_tile_pool(name="const", bufs=1)
        # persistent SBUF state
        xT = cpool.tile([128, cfg.NTP], f32, tag="xT")
        waug1 = cpool.tile([128, 390], f32, tag="waug1")
        waugn = cpool.tile([128, 390], f32, tag="waugn")
        wl3 = cpool.tile([128, 3, 128], f32, tag="wl3")
        blp1 = cpool.tile([128, 1], f32, tag="blp1")
        iota = cpool.tile([128, 128], DTT, tag="iota")
        ident = cpool.tile([128, 128], f32, tag="ident")
        dst3 = cpool.tile([128, TOT], f32, tag="dst3")
        zidx = cpool.tile([128, TOT * 8], dt.int16, tag="zidx")
        eidx = cpool.tile([128, TOT * 8], dt.int16, tag="eidx")

        nc.sync.dma_start(xT[:], xT0.ap())
        nc.sync.dma_start(waug1[:], waug1_d.ap())
        nc.sync.dma_start(waugn[:], waugn_d.ap())
        nc.sync.dma_start(wl3[:], wl3_d.ap().rearrange("k p m -> p k m"))
        nc.sync.dma_start(blp1[:], blp1_d.ap())
        nc.sync.dma_start(iota[:], iota_d.ap())
        nc.sync.dma_start(ident[:], ident_d.ap())
        nc.sync.dma_start(dst3[:], dst3_d.ap())
        nc.sync.dma_start(zidx[:], zidx_d.ap())
        nc.sync.dma_start(eidx[:], eidx_d.ap())

        psz_pool = tc.alloc_tile_pool(name="psz", bufs=3, space="PSUM")
        zst_pool = tc.alloc_tile_pool(name="zst", bufs=10)
        g_pool = tc.alloc_tile_pool(name="g", bufs=6)
        r_pool = tc.alloc_tile_pool(name="r", bufs=2)
        w_pool = tc.alloc_tile_pool(name="w", bufs=2)
        l_pool = tc.alloc_tile_pool(name="l", bufs=12)
        psb_pool = tc.alloc_tile_pool(name="psb", bufs=2, space="PSUM")
        pst_pool = tc.alloc_tile_pool(name="pst", bufs=1, space="PSUM")
        psx_pool = tc.alloc_tile_pool(name="psx", bufs=1, space="PSUM")
        s_pool = tc.alloc_tile_pool(name="s", bufs=2)
        a_pool = tc.alloc_tile_pool(name="a", bufs=2)
        at_pool = tc.alloc_tile_pool(name="at", bufs=2)
        x_pool = tc.alloc_tile_pool(name="x", bufs=2)
        pw_pool = tc.alloc_tile_pool(name="pw", bufs=2)
        pp_pool = tc.alloc_tile_pool(name="pp", bufs=1, space="PSUM")

        ps_pool_acc = pp_pool.tile([128, 128], f32, tag="poolacc")

        for layer in range(3):
            wa = waug1 if layer == 0 else waugn
            # ---------------- dense phase: z_aug + er tables ----------------
            for t in range(NT):
                psz = psz_pool.tile([128, 390], f32, tag="psz")
                nc.tensor.matmul(psz[:], xT[:, t * 128:(t + 1) * 128], wa[:],
                                 start=True, stop=True)
                zt = zst_pool.tile([128, ZC], DTT, tag="zt")
                if "stage" not in skip:
                    nc.vector.memset(zt[:, 128:388:129], 1.0)
                    nc.vector.tensor_copy(zt[:, 0:128], psz[:, 0:128])
                    nc.vector.tensor_copy(zt[:, 129:257], psz[:, 128:256])
                    nc.vector.tensor_copy(zt[:, 258:386], psz[:, 256:384])
                    nc.vector.tensor_copy(zt[:, 387:393], psz[:, 384:390])
                nc.sync.dma_start(
                    zaug.ap()[t * 128:(t + 1) * 128, 0:393], zt[:, 0:393])

            # ---------------- edge phase ------------------------------------
            for b in range(NB):
                ncb = int(nc_b[b])
                c0 = int(cum[b])
                R = r_pool.tile([128, int(nc_b.max()), EC], DTT, tag="R")
                psb = psb_pool.tile([128, 387], f32, tag="psb")
                for g0 in range(0, ncb, GC):
                    gsz = min(GC, ncb - g0)
                    Gt = g_pool.tile([128, GC, ZC], DTT, tag="G")
                    if "gather" not in skip: nc.gpsimd.dma_gather(
                        Gt[:, 0:gsz, :], zaug.ap(),
                        zidx[:, 8 * (c0 + g0): 8 * (c0 + g0 + gsz)],
                        num_idxs=gsz * 128, num_idxs_reg=gsz * 128,
                        elem_size=ZC, elem_step=ZC)
                    nc.gpsimd.dma_gather(
                        R[:, g0:g0 + gsz, :], zaug.ap()[:, 384:384 + EC],
                        eidx[:, 8 * (c0 + g0): 8 * (c0 + g0 + gsz)],
                        num_idxs=gsz * 128, num_idxs_reg=gsz * 128,
                        elem_size=EC, elem_step=ZC)
                    wt = w_pool.tile([128, GC, 3], f32, tag="wt")
                    nc.vector.tensor_tensor(
                        wt[:, 0:gsz, :], Gt[:, 0:gsz, EL0:EL0 + 3],
                        R[:, g0:g0 + gsz, 6:9], Alu.add)
                    nc.vector.scalar_tensor_tensor(
                        wt[:, 0:gsz, :], wt[:, 0:gsz, :], cfg.neg_slope,
                        wt[:, 0:gsz, :], Alu.mult, Alu.max)
                    nc.scalar.activation(wt[:, 0:gsz, :], wt[:, 0:gsz, :],
                                         Act.Exp)
                    for c in range(gsz):
                        cc = c0 + g0 + c
                        for h in range(3):
                            lh = l_pool.tile([128, 128], DTT, tag="lh")
                            if "onehot" not in skip: nc.vector.tensor_scalar(
                                lh[:], iota[:],
                                dst3[:, cc:cc + 1],
                                wt[:, c, h:h + 1].opt(),
                                Alu.is_equal, Alu.mult)
                            if "emm" in skip: continue
                            nc.tensor.matmul(
                                psb[:, 129 * h:129 * h + 129], lh[:],
                                Gt[:, c, 129 * h:129 * h + 129].opt(),
                                start=(g0 + c == 0 and h == 0),
                                stop=(g0 + c == ncb - 1 and h == 2))
                # normalize + transpose + linear
                r3 = s_pool.tile([128, 3], f32, tag="r3")
                nc.vector.reciprocal(r3[:], psb[:, 128:387:129])
                agg = a_pool.tile([128, 384], f32, tag="agg")
                for h in range(3):
                    nc.vector.tensor_scalar(
                        agg[:, 128 * h:128 * (h + 1)],
                        psb[:, 129 * h:129 * h + 128],
                        r3[:, h:h + 1], None, Alu.mult)
                aggT = at_pool.tile([128, 3, 128], f32, tag="aggT")
                for k in range(3):
                    pst = pst_pool.tile([128, 128], f32, tag="pst")
                    nc.tensor.transpose(pst[:], agg[:, 128 * k:128 * (k + 1)],
                                        ident[:])
                    nc.vector.tensor_copy(aggT[:, k, :].opt(), pst[:])
                bw = min(128, SH - b * 128)
                if layer < 2:
                    psx = psx_pool.tile([128, 128], f32, tag="psx")
                    for k in range(3):
                        nc.tensor.matmul(psx[:], wl3[:, k, :].opt(),
                                         aggT[:, k, :].opt(),
                                         start=(k == 0), stop=(k == 2))
                    xsb = x_pool.tile([128, 128], f32, tag="xsb")
                    nc.vector.tensor_scalar(xsb[:], psx[:], blp1[:], None,
                                            Alu.add)
                    if b < HB:
                        nc.sync.dma_start(
                            xsh1.ap()[:, b * 128:b * 128 + bw], xsb[:, 0:bw])
                    else:
                        o = b * 128 - SH1
                        nc.sync.dma_start(
                            xsh2.ap()[:, o:o + bw], xsb[:, 0:bw])
                    if b == HB - 1:
                        do_gather(nc, 0, timing_mode)
                else:
                    psx = psx_pool.tile([128, 128], f32, tag="psx")
                    for k in range(3):
                        nc.tensor.matmul(psx[:], aggT[:, k, :].opt(),
                                         wl3[:, k, :].opt(),
                                         start=(k == 0), stop=(k == 2))
                    x3 = x_pool.tile([128, 128], f32, tag="xsb")
                    nc.vector.tensor_copy(x3[:], psx[:])
                    pw = pw_pool.tile([128, 128], f32, tag="pw")
                    nc.sync.dma_start(pw[:], poolw_d.ap()[b])
                    nc.tensor.matmul(ps_pool_acc[:], pw[:], x3[:],
                                     start=(b == 0), stop=(b == NB - 1))

            if layer < 2:
                do_gather(nc, 1, timing_mode)
                for j in range(gpb):
                    nc.sync.dma_start(xT[:, j * SH:j * SH + SH1],
                                      xgath1.ap()[j])
                    nc.sync.dma_start(xT[:, j * SH + SH1:(j + 1) * SH],
                                      xgath2.ap()[j])
                if layer == 0:
                    nc.sync.dma_start(blp1[:], blpn_d.ap())

        po = x_pool.tile([128, 128], f32, tag="po")
        nc.vector.tensor_copy(po[:], ps_pool_acc[:])
        nc.sync.dma_start(pool_out.ap(), po[:])

        for p in (pp_pool, pw_pool, x_pool, at_pool, a_pool, s_pool,
                  psx_pool, pst_pool, psb_pool, l_pool, w_pool, r_pool,
                  g_pool, zst_pool, psz_pool, cpool):
            p.release()

    nc.compile()
    return nc


# ---------------------------------------------------------------------------
# top-level kernel
# ---------------------------------------------------------------------------

def _prepare(cfg, inputs):
    """Returns (nc_b, in_maps, host_meta)."""
    npf = np.asarray
    per_core_edges = []
    nc_b = np.zeros(cfg.NB, np.int64)
    for br, (s, d) in enumerate((("srcA", "dstA"), ("srcB", "dstB"))):
        src = npf(inputs[s]).astype(np.int64)
        dst = npf(inputs[d]).astype(np.int64)
        for q in range(cfg.gpb):
            es, ed, cnt, lo = _prep_edges(cfg, src, dst, q)
            per_core_edges.append((es, ed, lo))
            nc_b = np.maximum(nc_b, -(-cnt // 128))
    in_maps = []
    host_meta = {}
    if cfg.table_bf16:
        import ml_dtypes
        tdt = ml_dtypes.bfloat16
    else:
        tdt = np.float32
    iota = np.tile(np.arange(128, dtype=tdt), (128, 1))
    ident = np.eye(128, dtype=np.float32)
    for br in range(2):
        sfx = "AB"[br]
        W1 = npf(inputs["W1" + sfx]); al1 = npf(inputs["al1" + sfx])
        ar1 = npf(inputs["ar1" + sfx]); b1 = npf(inputs["b1" + sfx])
        Wn = npf(inputs["Wn" + sfx]); aln = npf(inputs["aln" + sfx])
        arn = npf(inputs["arn" + sfx]); bn = npf(inputs["bn" + sfx])
        Wl = npf(inputs["Wl" + sfx]); bl = npf(inputs["bl" + sfx])
        gid = npf(inputs["gid" + sfx]).astype(np.int64)
        feats = npf(inputs["feats" + sfx]).astype(np.float32)
        waug1, waugn, wl3, blp1, blpn = _prep_branch_weights(
            cfg, W1, al1, ar1, b1, Wn, aln, arn, bn, Wl, bl)
        xT0 = np.zeros((128, cfg.NTP), np.float32)
        xT0[:, :cfg.N] = feats.T
        host_meta[sfx] = dict(blpn=blpn, gid=gid)
        for q in range(cfg.gpb):
            es, ed, lo = per_core_edges[br * cfg.gpb + q]
            zidx, eidx, dst3 = _pack_core(cfg, es, ed, lo, nc_b)
            poolw = np.zeros((cfg.NB, 128, 128), np.float32)
            for b in range(cfg.NB):
                for i in range(min(128, cfg.SH - b * 128)):
                    n = lo + b * 128 + i
                    if n < cfg.N:
                        poolw[b, i, gid[n]] = 1.0
            in_maps.append({
                "xT0": xT0, "waug1": waug1, "waugn": waugn,
                "wl3": wl3, "blp1": blp1.reshape(128, 1),
                "blpn": blpn.reshape(128, 1),
                "iota": iota, "ident": ident,
                "dst3": dst3, "zidx": zidx, "eidx": eidx, "poolw": poolw,
            })
    return nc_b, in_maps, host_meta


def _finalize(cfg, inputs, host_meta, pool_outs):
    """pool_outs: list of 8 [128,128] arrays -> full output [G,1] float64."""
    out = {}
    for br in range(2):
        sfx = "AB"[br]
        total = np.zeros((128, 128), np.float64)
        for q in range(cfg.gpb):
            total += pool_outs[br * cfg.gpb + q].astype(np.float64)
        gid = host_meta[sfx]["gid"]
        cnt = np.bincount(gid, minlength=128).astype(np.float64)
        total += cnt[:, None] * host_meta[sfx]["blpn"].astype(np.float64)[None, :]
        out[sfx] = (total / np.maximum(cnt[:, None], 1.0))[:cfg.G]
    cat = np.concatenate([out["A"], out["B"]], axis=1)
    Wo = np.asarray(inputs["Wo"]).astype(np.float64)
    bo = np.asarray(inputs["bo"]).astype(np.float64)
    return (cat @ Wo + bo).astype(np.float64)


_CACHE = {}


def kernel(**inputs):
    cfg = Cfg(N=inputs["featsA"].shape[0], G=128)
    nc_b, in_maps, host_meta = _prepare(cfg, inputs)
    key = ("prog", tuple(nc_b.tolist()), cfg.table_bf16)
    if key not in _CACHE:
        _CACHE[key] = build_program(cfg, nc_b)
    nc = _CACHE[key]
    from concourse.bass_utils import run_bass_kernel_spmd
    res = run_bass_kernel_spmd(nc, in_maps, list(range(cfg.n_cores)))
    pool_outs = [r["pool_out"] for r in res.results]
    return _finalize(cfg, inputs, host_meta, pool_outs)



# revision 20
# speedup vs baseline: 1.3781x; 1.3781x over previous
"""Trainium2 Bass kernel for nn_DoubleNet (two GATNet branches + avg-pool + linear).

Strategy (8 NeuronCores), "x-gather" design:
  - Cores 0-3 run branch A, cores 4-7 run branch B (same SPMD program,
    different data). Within a branch, dst nodes are sharded across 4 cores.
  - Key algebraic refactor: for one GAT layer followed by the shared linear,
      x_next[d] = sum_h (sum_e a_e^h x[src_e]) @ M_h + blp,   M_h = W_h @ Wl_h
    so the edge phase only needs x[src] (128 cols) per edge instead of
    z[src] (384 cols), and the per-head linear maps are applied AFTER
    aggregation, per 128-dst block. el/er attention scalars are linear in x
    too (el = x @ (W_h al_h)), kept in the per-node table row.
  - Per-layer node table (DRAM, bf16 rows of 512B):
      row(n) = [x (128) | el (3) | er (3) | 1 | pad] ; row id = q*5120 + r.
    Layer-0 table is host-built from feats; later tables are written
    per-block by the epilogue and all-gathered across the branch's 4 cores
    in two halves for overlap.
  - Edge phase per dst block: ONE merged dma_gather call pulls, per 128-edge
    chunk, 128 src rows and 128 dst rows (interleaved slabs; er comes from
    the dst rows). w = exp(leaky(el+er)); per head a w-scaled one-hot matmul
    scatter-adds messages+denominator into PSUM ([x|...|1] rhs, 135 cols).
  - Epilogue per block: normalize per head (Act copy w/ scale), PE-transpose,
    3 matmuls against [M_h | M_h@Cnext] (134 cols) -> x_next and next-layer
    el/er in one shot; bias-add; row-write to the next table (layers 0,1) or
    one-hot pool matmul (layer 2). Host divides pooled sums by graph counts
    and applies the output linear.
"""

import sys

sys.path.insert(0, "/opt/trn_rl_repo")

import numpy as np


# ---------------------------------------------------------------------------
# configuration
# ---------------------------------------------------------------------------

class Cfg:
    def __init__(self, N=20000, G=128, H=3, EMB=128, F=128, n_cores=8,
                 neg_slope=0.2):
        assert F == 128 and EMB == 128 and H == 3
        self.N, self.G, self.H, self.EMB, self.F = N, G, H, EMB, F
        self.n_cores = n_cores
        self.gpb = n_cores // 2            # cores per branch
        assert N % self.gpb == 0
        self.SH = N // self.gpb            # dst nodes per core
        self.NB = -(-self.SH // 128)       # dst blocks per core
        self.SHP = self.NB * 128           # padded rows per shard
        self.NTOT = self.gpb * self.SHP    # table rows
        self.HB = self.NB // 2
        self.neg_slope = neg_slope
        # table row layout (bf16 cols): [x 0:128 | el 128:131 | er 131:134 |
        #   one @134 | pad to 256]. er window = cols 128:256 of the row.
        self.ROW = 256
        self.EL0, self.ER0, self.ONE = 128, 131, 134
        self.RHS = 135                     # matmul rhs cols [x|el|er|1]
        self.GCH = 4                       # chunks per gather call (256 idx each)
        self.dma_scratch = 16384           # SWDGE ring size driver


# ---------------------------------------------------------------------------
# host-side data prep
# ---------------------------------------------------------------------------

def _prep_edges(cfg, src, dst, q):
    """Edges of one core (dst in its shard), dst-sorted, fake rows added."""
    lo = q * cfg.SH
    sel = (dst >= lo) & (dst < lo + cfg.SH)
    es = src[sel].astype(np.int64)
    ed = (dst[sel].astype(np.int64) - lo)
    nfake = cfg.SHP - cfg.SH
    if nfake:
        es = np.concatenate([es, np.zeros(nfake, np.int64)])
        ed = np.concatenate([ed, np.arange(cfg.SH, cfg.SHP, dtype=np.int64)])
    order = np.argsort(ed, kind="stable")
    es, ed = es[order], ed[order]
    cnt = np.bincount(ed // 128, minlength=cfg.NB)
    return es, ed, cnt


def _row_of(cfg, n):
    """Global node id -> table row id.

    Rows are grouped (half, rank, local) with HROW = HB*128 rows per rank per
    half, so each all-gather half lands in one contiguous table region.
    """
    q, r = n // cfg.SH, n % cfg.SH
    HROW = cfg.HB * 128
    return np.where(r < HROW, q * HROW + r,
                    cfg.gpb * HROW + q * HROW + (r - HROW))


def _pack_core(cfg, es, ed, q, nc_b):
    """Merged (src,dst) interleaved index array + dst3 per chunk."""
    TOT = int(nc_b.sum())
    idx = np.zeros(TOT * 256, np.int16)    # per chunk: 128 src rows, 128 dst
    dst3 = np.full(TOT * 128, -1.0, np.float32)
    epos = np.searchsorted(ed, np.arange(0, cfg.SHP + 1, 128))
    c0 = 0
    for b in range(cfg.NB):
        s, e = epos[b], epos[b + 1]
        srows = _row_of(cfg, es[s:e])
        HROW = cfg.HB * 128
        dl = ed[s:e]
        drows = np.where(dl < HROW, q * HROW + dl,
                         cfg.gpb * HROW + q * HROW + (dl - HROW))
        off = ed[s:e] - b * 128
        for c in range(int(nc_b[b])):
            o, n = c * 128, min(128, (e - s) - c * 128)
            if n <= 0:
                break
            cc = c0 + c
            idx[cc * 256: cc * 256 + n] = srows[o:o + n]
            idx[cc * 256 + 128: cc * 256 + 128 + n] = drows[o:o + n]
            dst3[cc * 128: cc * 128 + n] = off[o:o + n].astype(np.float32)
        c0 += int(nc_b[b])

    def wrap(a):  # flat i -> (partition i%16, col i//16), replicated to 128
        return np.tile(a.reshape(-1, 16).T, (8, 1)).copy()

    d3 = dst3.reshape(TOT, 128).T.copy()
    return wrap(idx), d3


def _branch_consts(cfg, W1, al1, ar1, b1, Wn, aln, arn, bn, Wl, bl):
    """Per-branch folded weights: M|MC rhs tensors, bias tiles, el/er coeffs."""
    H, EMB = cfg.H, cfg.EMB

    def coeffs(W, al, ar):
        C = np.zeros((W.shape[0], 6), np.float64)
        for h in range(H):
            Wh = W[:, h * EMB:(h + 1) * EMB].astype(np.float64)
            C[:, h] = Wh @ al[h].astype(np.float64)
            C[:, 3 + h] = Wh @ ar[h].astype(np.float64)
        return C

    Wl64 = Wl.astype(np.float64)
    C1 = coeffs(W1, al1, ar1)
    Cn = coeffs(Wn, aln, arn)

    def rhsW(W):
        R = np.zeros((H, W.shape[0], 134), np.float32)
        for h in range(H):
            Wh = W[:, h * EMB:(h + 1) * EMB].astype(np.float64)
            M = Wh @ Wl64[h * EMB:(h + 1) * EMB, :]
            R[h, :, 0:128] = M.astype(np.float32)
            R[h, :, 128:134] = (M @ Cn).astype(np.float32)
        return R

    blp1 = (b1.astype(np.float64) @ Wl64 + bl.astype(np.float64))
    blpn = (bn.astype(np.float64) @ Wl64 + bl.astype(np.float64))

    def btile(blp):
        B = np.zeros((134,), np.float32)
        B[0:128] = blp.astype(np.float32)
        B[128:134] = (blp @ Cn).astype(np.float32)
        return np.tile(B[None, :], (128, 1)).copy()

    return rhsW(W1), rhsW(Wn), btile(blp1), btile(blpn), C1


def _table0(cfg, feats, C1, tdt):
    """Host-built layer-0 table [NTOT, 256] bf16 ((half, rank, local) rows)."""
    t = np.zeros((cfg.NTOT, cfg.ROW), np.float32)
    f64 = feats.astype(np.float64)
    elr = (f64 @ C1).astype(np.float32)
    rows = _row_of(cfg, np.arange(cfg.N))
    t[rows, 0:128] = feats
    t[rows, 128:134] = elr
    t[:, cfg.ONE] = 1.0
    return t.astype(tdt)


# ---------------------------------------------------------------------------
# device program
# ---------------------------------------------------------------------------

def build_program(cfg, nc_b, timing_mode=False, skip=()):
    import concourse.bass as bass
    import concourse.mybir as mybir
    import concourse.tile as tile
    from concourse import bacc

    dt = mybir.dt
    f32 = dt.float32
    bf16 = dt.bfloat16
    Alu = mybir.AluOpType
    Act = mybir.ActivationFunctionType

    NB, HB, SH, SHP = cfg.NB, cfg.HB, cfg.SH, cfg.SHP
    ROW, RHS = cfg.ROW, cfg.RHS
    TOT = int(nc_b.sum())
    NCMAX = int(nc_b.max())
    cum = np.concatenate([[0], np.cumsum(nc_b)]).astype(int)
    gpb = cfg.gpb
    groups = [list(range(gpb)), list(range(gpb, 2 * gpb))]

    nc = bacc.Bacc("TRN2", target_bir_lowering=False, debug=False,
                   num_devices=cfg.n_cores,
                   dynamic_dma_scratch_size=cfg.dma_scratch)

    # inputs -----------------------------------------------------------------
    t0_d = nc.dram_tensor("t0", [cfg.NTOT, ROW], bf16, kind="ExternalInput")
    rhs1_d = nc.dram_tensor("rhs1", [3, 128, 134], bf16, kind="ExternalInput")
    rhsn_d = nc.dram_tensor("rhsn", [3, 128, 134], bf16, kind="ExternalInput")
    bt1_d = nc.dram_tensor("bt1", [128, 134], f32, kind="ExternalInput")
    btn_d = nc.dram_tensor("btn", [128, 134], f32, kind="ExternalInput")
    iota_d = nc.dram_tensor("iota", [128, 128], bf16, kind="ExternalInput")
    ident_d = nc.dram_tensor("ident", [128, 128], f32, kind="ExternalInput")
    dst3_d = nc.dram_tensor("dst3", [128, TOT], f32, kind="ExternalInput")
    idx_d = nc.dram_tensor("idx", [128, TOT * 16], dt.int16,
                           kind="ExternalInput")
    poolw_d = nc.dram_tensor("poolw", [NB, 128, 128], bf16,
                             kind="ExternalInput")
    pool_out = nc.dram_tensor("pool_out", [128, 128], f32,
                              kind="ExternalOutput")

    # internal DRAM ----------------------------------------------------------
    ti_d = nc.dram_tensor("ti", [cfg.NTOT, ROW], bf16)  # layer-0 table copy
    t1_d = nc.dram_tensor("t1", [cfg.NTOT, ROW], bf16)
    t2_d = nc.dram_tensor("t2", [cfg.NTOT, ROW], bf16)
    HROW = HB * 128
    # standalone collective buffers (whole tensors; CC-safe like baseline)
    tsh_d = [[nc.dram_tensor(f"tsh{l}{h}", [HROW, ROW], bf16)
              for h in range(2)] for l in range(2)]
    tg_d = [[nc.dram_tensor(f"tg{l}{h}", [gpb, HROW, ROW], bf16)
             for h in range(2)] for l in range(2)]

    def do_gather(layer, tfull, half):
        """All-gather one half of the shard table into the full table."""
        tsh, tg = tsh_d[layer][half], tg_d[layer][half]
        if timing_mode:
            for j in range(gpb):
                nc.sync.dma_start(tg.ap()[j], tsh.ap())
        else:
            nc.gpsimd.collective_compute(
                "AllGather", mybir.AluOpType.bypass, replica_groups=groups,
                ins=[tsh.ap()], outs=[tg.ap()])
        nc.sync.dma_start(
            tfull.ap()[half * gpb * HROW:(half + 1) * gpb * HROW, :],
            tg.ap().rearrange("q r c -> (q r) c"))

    with tile.TileContext(nc) as tc:
        cpool = tc.alloc_tile_pool(name="const", bufs=1)
        rhs1 = cpool.tile([128, 3, 134], bf16, tag="rhs1")
        rhsn = cpool.tile([128, 3, 134], bf16, tag="rhsn")
        bt1 = cpool.tile([128, 134], f32, tag="bt1")
        btn = cpool.tile([128, 134], f32, tag="btn")
        iota = cpool.tile([128, 128], bf16, tag="iota")
        identf = cpool.tile([128, 128], f32, tag="identf")
        dst3 = cpool.tile([128, TOT], f32, tag="dst3")
        idx = cpool.tile([128, TOT * 16], dt.int16, tag="idx")

        nc.sync.dma_start(rhs1[:], rhs1_d.ap().rearrange("k p m -> p k m"))
        nc.sync.dma_start(rhsn[:], rhsn_d.ap().rearrange("k p m -> p k m"))
        nc.sync.dma_start(bt1[:], bt1_d.ap())
        nc.sync.dma_start(btn[:], btn_d.ap())
        nc.sync.dma_start(iota[:], iota_d.ap())
        nc.sync.dma_start(identf[:], ident_d.ap())
        nc.sync.dma_start(dst3[:], dst3_d.ap())
        nc.sync.dma_start(idx[:], idx_d.ap())
        # gather sources must be internal DRAM; stage the input table once
        nc.sync.dma_start(ti_d.ap(), t0_d.ap())

        g_pool = tc.alloc_tile_pool(name="g", bufs=2)
        w_pool = tc.alloc_tile_pool(name="w", bufs=2)
        l_pool = tc.alloc_tile_pool(name="l", bufs=12)
        psb_pool = tc.alloc_tile_pool(name="psb", bufs=2, space="PSUM")
        s_pool = tc.alloc_tile_pool(name="s", bufs=2)
        u_pool = tc.alloc_tile_pool(name="u", bufs=6)
        pst_pool = tc.alloc_tile_pool(name="pst", bufs=2, space="PSUM")
        ut_pool = tc.alloc_tile_pool(name="ut", bufs=6)
        psx_pool = tc.alloc_tile_pool(name="psx", bufs=2, space="PSUM")
        x_pool = tc.alloc_tile_pool(name="x", bufs=3)
        pw_pool = tc.alloc_tile_pool(name="pw", bufs=2)
        pp_pool = tc.alloc_tile_pool(name="pp", bufs=1, space="PSUM")

        ps_pool_acc = pp_pool.tile([128, 128], f32, tag="poolacc")

        for layer in range(3):
            tbl = (ti_d, t1_d, t2_d)[layer]
            rw = rhs1 if layer == 0 else rhsn
            bt = bt1 if layer == 0 else btn
            for b in range(NB):
                ncb = int(nc_b[b])
                c0 = int(cum[b])
                # ---- merged gather: per chunk 128 src rows + 128 dst rows
                Gt = g_pool.tile([128, 2 * NCMAX, ROW], bf16, tag="G")
                if "gather" not in skip:
                    for g0 in range(0, ncb, cfg.GCH):
                        gsz = min(cfg.GCH, ncb - g0)
                        nc.gpsimd.dma_gather(
                            Gt[:, 2 * g0:2 * (g0 + gsz), :], tbl.ap(),
                            idx[:, 16 * (c0 + g0): 16 * (c0 + g0 + gsz)],
                            num_idxs=gsz * 256, num_idxs_reg=gsz * 256,
                            elem_size=ROW, elem_step=ROW)
                # ---- attention weights w = exp(leaky(el_src + er_dst))
                wt = w_pool.tile([128, NCMAX, 3], f32, tag="wt")
                nc.vector.tensor_tensor(
                    wt[:, 0:ncb, :], Gt[:, 0:2 * ncb:2, cfg.EL0:cfg.EL0 + 3],
                    Gt[:, 1:2 * ncb:2, cfg.ER0:cfg.ER0 + 3], Alu.add)
                nc.vector.scalar_tensor_tensor(
                    wt[:, 0:ncb, :], wt[:, 0:ncb, :], cfg.neg_slope,
                    wt[:, 0:ncb, :], Alu.mult, Alu.max)
                nc.scalar.activation(wt[:, 0:ncb, :], wt[:, 0:ncb, :],
                                     Act.Exp)
                # ---- scatter-add via w-scaled one-hot matmuls
                psb = psb_pool.tile([128, 3 * RHS], f32, tag="psb")
                for c in range(ncb):
                    cc = c0 + c
                    for h in range(3):
                        lh = l_pool.tile([128, 128], bf16, tag="lh")
                        if "onehot" not in skip:
                            nc.vector.tensor_scalar(
                                lh[:], iota[:], dst3[:, cc:cc + 1],
                                wt[:, c, h:h + 1].opt(),
                                Alu.is_equal, Alu.mult)
                        if "emm" in skip:
                            continue
                        nc.tensor.matmul(
                            psb[:, RHS * h:RHS * h + RHS], lh[:],
                            Gt[:, 2 * c, 0:RHS].opt(),
                            start=(c == 0 and h == 0),
                            stop=(c == ncb - 1 and h == 2))
                # ---- epilogue: normalize, transpose, apply M|MC
                r3 = s_pool.tile([128, 3], f32, tag="r3")
                nc.vector.reciprocal(r3[:], psb[:, RHS - 1::RHS])
                psx = psx_pool.tile([128, 134], f32, tag="psx")
                for h in range(3):
                    u = u_pool.tile([128, 128], f32, tag="u")
                    nc.scalar.activation(u[:], psb[:, RHS * h:RHS * h + 128],
                                         Act.Copy, scale=r3[:, h:h + 1])
                    pst = pst_pool.tile([128, 128], f32, tag="pst")
                    nc.tensor.transpose(pst[:], u[:], identf[:])
                    uT = ut_pool.tile([128, 128], bf16, tag="uT")
                    nc.scalar.activation(uT[:], pst[:], Act.Copy)
                    nc.tensor.matmul(psx[:], uT[:], rw[:, h, :].opt(),
                                     start=(h == 0), stop=(h == 2))
                xsb = x_pool.tile([128, ROW], bf16, tag="xsb")
                nc.vector.tensor_tensor(xsb[:, 0:134], psx[:], bt[:], Alu.add)
                if layer < 2:
                    nc.vector.memset(xsb[:, cfg.ONE:ROW], 1.0)
                    half, bh = (0, b) if b < HB else (1, b - HB)
                    nc.sync.dma_start(
                        tsh_d[layer][half].ap()[bh * 128:(bh + 1) * 128, :],
                        xsb[:])
                    if b == HB - 1:
                        do_gather(layer, (t1_d, t2_d)[layer], 0)
                    elif b == NB - 1:
                        do_gather(layer, (t1_d, t2_d)[layer], 1)
                else:
                    pw = pw_pool.tile([128, 128], bf16, tag="pw")
                    nc.sync.dma_start(pw[:], poolw_d.ap()[b])
                    nc.tensor.matmul(ps_pool_acc[:], pw[:], xsb[:, 0:128],
                                     start=(b == 0), stop=(b == NB - 1))

        po = x_pool.tile([128, 128], f32, tag="po")
        nc.vector.tensor_copy(po[:], ps_pool_acc[:])
        nc.sync.dma_start(pool_out.ap(), po[:])

        for p in (pp_pool, pw_pool, x_pool, psx_pool, ut_pool, pst_pool,
                  u_pool, s_pool, psb_pool, l_pool, w_pool, g_pool, cpool):
            p.release()

    nc.compile()
    return nc


# ---------------------------------------------------------------------------
# top-level kernel
# ---------------------------------------------------------------------------

def _prepare(cfg, inputs):
    """Returns (nc_b, in_maps, host_meta)."""
    npf = np.asarray
    import ml_dtypes
    tdt = ml_dtypes.bfloat16

    per_core_edges = []
    nc_b = np.zeros(cfg.NB, np.int64)
    for br, (s, d) in enumerate((("srcA", "dstA"), ("srcB", "dstB"))):
        src = npf(inputs[s]).astype(np.int64)
        dst = npf(inputs[d]).astype(np.int64)
        for q in range(cfg.gpb):
            es, ed, cnt = _prep_edges(cfg, src, dst, q)
            per_core_edges.append((es, ed))
            nc_b = np.maximum(nc_b, -(-cnt // 128))

    iota = np.tile(np.arange(128, dtype=tdt), (128, 1))
    ident = np.eye(128, dtype=np.float32)

    in_maps = []
    host_meta = {}
    for br in range(2):
        sfx = "AB"[br]
        W1 = npf(inputs["W1" + sfx]); al1 = npf(inputs["al1" + sfx])
        ar1 = npf(inputs["ar1" + sfx]); b1 = npf(inputs["b1" + sfx])
        Wn = npf(inputs["Wn" + sfx]); aln = npf(inputs["aln" + sfx])
        arn = npf(inputs["arn" + sfx]); bn = npf(inputs["bn" + sfx])
        Wl = npf(inputs["Wl" + sfx]); bl = npf(inputs["bl" + sfx])
        gid = npf(inputs["gid" + sfx]).astype(np.int64)
        feats = npf(inputs["feats" + sfx]).astype(np.float32)
        rhs1, rhsn, bt1, btn, C1 = _branch_consts(
            cfg, W1, al1, ar1, b1, Wn, aln, arn, bn, Wl, bl)
        t0 = _table0(cfg, feats, C1, tdt)
        host_meta[sfx] = dict(gid=gid)
        for q in range(cfg.gpb):
            es, ed = per_core_edges[br * cfg.gpb + q]
            idx, d3 = _pack_core(cfg, es, ed, q, nc_b)
            poolw = np.zeros((cfg.NB, 128, 128), tdt)
            for b in range(cfg.NB):
                for i in range(min(128, cfg.SH - b * 128)):
                    n = q * cfg.SH + b * 128 + i
                    if n < cfg.N:
                        poolw[b, i, gid[n]] = 1.0
            in_maps.append({
                "t0": t0,
                "rhs1": rhs1.astype(tdt), "rhsn": rhsn.astype(tdt),
                "bt1": bt1, "btn": btn,
                "iota": iota, "ident": ident,
                "dst3": d3, "idx": idx, "poolw": poolw,
            })
    return nc_b, in_maps, host_meta


def _finalize(cfg, inputs, host_meta, pool_outs):
    """pool_outs: list of 8 [128,128] arrays -> full output [G,1] float64."""
    out = {}
    for br in range(2):
        sfx = "AB"[br]
        total = np.zeros((128, 128), np.float64)
        for q in range(cfg.gpb):
            total += pool_outs[br * cfg.gpb + q].astype(np.float64)
        gid = host_meta[sfx]["gid"]
        cnt = np.bincount(gid, minlength=128).astype(np.float64)
        out[sfx] = (total / np.maximum(cnt[:, None], 1.0))[:cfg.G]
    cat = np.concatenate([out["A"], out["B"]], axis=1)
    Wo = np.asarray(inputs["Wo"]).astype(np.float64)
    bo = np.asarray(inputs["bo"]).astype(np.float64)
    return (cat @ Wo + bo).astype(np.float64)


_CACHE = {}


def kernel(**inputs):
    cfg = Cfg(N=inputs["featsA"].shape[0], G=128)
    nc_b, in_maps, host_meta = _prepare(cfg, inputs)
    key = ("prog", tuple(nc_b.tolist()))
    if key not in _CACHE:
        _CACHE[key] = build_program(cfg, nc_b)
    nc = _CACHE[key]
    from concourse.bass_utils import run_bass_kernel_spmd
    res = run_bass_kernel_spmd(nc, in_maps, list(range(cfg.n_cores)))
    pool_outs = [r["pool_out"] for r in res.results]
    return _finalize(cfg, inputs, host_meta, pool_outs)


# revision 25
# speedup vs baseline: 1.4707x; 1.0672x over previous
"""Trainium2 Bass kernel for nn_DoubleNet (two GATNet branches + avg-pool + linear).

Strategy (8 NeuronCores), "x-gather" design:
  - Cores 0-3 run branch A, cores 4-7 run branch B (same SPMD program,
    different data). Within a branch, dst nodes are sharded across 4 cores.
  - Key algebraic refactor: for one GAT layer followed by the shared linear,
      x_next[d] = sum_h (sum_e a_e^h x[src_e]) @ M_h + blp,   M_h = W_h @ Wl_h
    so the edge phase only needs x[src] (128 cols) per edge instead of
    z[src] (384 cols), and the per-head linear maps are applied AFTER
    aggregation, per 128-dst block. el/er attention scalars are linear in x
    too (el = x @ (W_h al_h)), kept in the per-node table row.
  - Per-layer node table (DRAM, bf16 rows of 512B):
      row(n) = [x (128) | el (3) | er (3) | 1 | pad] ; row id = q*5120 + r.
    Layer-0 table is host-built from feats; later tables are written
    per-block by the epilogue and all-gathered across the branch's 4 cores
    in two halves for overlap.
  - Edge phase per dst block: ONE merged dma_gather call pulls, per 128-edge
    chunk, 128 src rows and 128 dst rows (interleaved slabs; er comes from
    the dst rows). w = exp(leaky(el+er)); per head a w-scaled one-hot matmul
    scatter-adds messages+denominator into PSUM ([x|...|1] rhs, 135 cols).
  - Epilogue per block: normalize per head (Act copy w/ scale), PE-transpose,
    3 matmuls against [M_h | M_h@Cnext] (134 cols) -> x_next and next-layer
    el/er in one shot; bias-add; row-write to the next table (layers 0,1) or
    one-hot pool matmul (layer 2). Host divides pooled sums by graph counts
    and applies the output linear.
"""

import sys

sys.path.insert(0, "/opt/trn_rl_repo")

import numpy as np


# ---------------------------------------------------------------------------
# configuration
# ---------------------------------------------------------------------------

class Cfg:
    def __init__(self, N=20000, G=128, H=3, EMB=128, F=128, n_cores=8,
                 neg_slope=0.2):
        assert F == 128 and EMB == 128 and H == 3
        self.N, self.G, self.H, self.EMB, self.F = N, G, H, EMB, F
        self.n_cores = n_cores
        self.gpb = n_cores // 2            # cores per branch
        assert N % self.gpb == 0
        self.SH = N // self.gpb            # dst nodes per core
        self.NB = -(-self.SH // 128)       # dst blocks per core
        self.SHP = self.NB * 128           # padded rows per shard
        self.NTOT = self.gpb * self.SHP    # table rows
        self.HB = self.NB // 2
        self.neg_slope = neg_slope
        # table row layout (bf16 cols): [x 0:128 | el 128:131 | er 131:134 |
        #   one @134 | pad to 256]. er window = cols 128:256 of the row.
        self.ROW = 256
        self.EL0, self.ER0, self.ONE = 128, 131, 134
        self.RHS = 135                     # matmul rhs cols [x|el|er|1]
        self.GCH = 4                       # chunks per gather call (256 idx each)
        self.dma_scratch = 32768           # SWDGE ring size driver


# ---------------------------------------------------------------------------
# host-side data prep
# ---------------------------------------------------------------------------

def _prep_edges(cfg, src, dst, q):
    """Edges of one core (dst in its shard), dst-sorted, fake rows added."""
    lo = q * cfg.SH
    sel = (dst >= lo) & (dst < lo + cfg.SH)
    es = src[sel].astype(np.int64)
    ed = (dst[sel].astype(np.int64) - lo)
    nfake = cfg.SHP - cfg.SH
    if nfake:
        es = np.concatenate([es, np.zeros(nfake, np.int64)])
        ed = np.concatenate([ed, np.arange(cfg.SH, cfg.SHP, dtype=np.int64)])
    order = np.argsort(ed, kind="stable")
    es, ed = es[order], ed[order]
    cnt = np.bincount(ed // 128, minlength=cfg.NB)
    return es, ed, cnt


def _row_of(cfg, n):
    """Global node id -> table row id.

    Rows are grouped (half, rank, local) with HROW = HB*128 rows per rank per
    half, so each all-gather half lands in one contiguous table region.
    """
    q, r = n // cfg.SH, n % cfg.SH
    HROW = cfg.HB * 128
    return np.where(r < HROW, q * HROW + r,
                    cfg.gpb * HROW + q * HROW + (r - HROW))


def _pack_core(cfg, es, ed, q, nc_b):
    """Merged (src,dst) interleaved index array + dst3 per chunk."""
    TOT = int(nc_b.sum())
    idx = np.zeros(TOT * 256, np.int16)    # per chunk: 128 src rows, 128 dst
    dst3 = np.full(TOT * 128, -1.0, np.float32)
    epos = np.searchsorted(ed, np.arange(0, cfg.SHP + 1, 128))
    c0 = 0
    for b in range(cfg.NB):
        s, e = epos[b], epos[b + 1]
        srows = _row_of(cfg, es[s:e])
        HROW = cfg.HB * 128
        dl = ed[s:e]
        drows = np.where(dl < HROW, q * HROW + dl,
                         cfg.gpb * HROW + q * HROW + (dl - HROW))
        off = ed[s:e] - b * 128
        for c in range(int(nc_b[b])):
            o, n = c * 128, min(128, (e - s) - c * 128)
            if n <= 0:
                break
            cc = c0 + c
            idx[cc * 256: cc * 256 + n] = srows[o:o + n]
            idx[cc * 256 + 128: cc * 256 + 128 + n] = drows[o:o + n]
            dst3[cc * 128: cc * 128 + n] = off[o:o + n].astype(np.float32)
        c0 += int(nc_b[b])

    def wrap(a):  # flat i -> (partition i%16, col i//16), replicated to 128
        return np.tile(a.reshape(-1, 16).T, (8, 1)).copy()

    d3 = dst3.reshape(TOT, 128).T.copy()
    return wrap(idx), d3


def _branch_consts(cfg, W1, al1, ar1, b1, Wn, aln, arn, bn, Wl, bl):
    """Per-branch folded weights: M|MC rhs tensors, bias tiles, el/er coeffs."""
    H, EMB = cfg.H, cfg.EMB

    def coeffs(W, al, ar):
        C = np.zeros((W.shape[0], 6), np.float64)
        for h in range(H):
            Wh = W[:, h * EMB:(h + 1) * EMB].astype(np.float64)
            C[:, h] = Wh @ al[h].astype(np.float64)
            C[:, 3 + h] = Wh @ ar[h].astype(np.float64)
        return C

    Wl64 = Wl.astype(np.float64)
    C1 = coeffs(W1, al1, ar1)
    Cn = coeffs(Wn, aln, arn)

    def rhsW(W):
        R = np.zeros((H, W.shape[0], 134), np.float32)
        for h in range(H):
            Wh = W[:, h * EMB:(h + 1) * EMB].astype(np.float64)
            M = Wh @ Wl64[h * EMB:(h + 1) * EMB, :]
            R[h, :, 0:128] = M.astype(np.float32)
            R[h, :, 128:134] = (M @ Cn).astype(np.float32)
        return R

    blp1 = (b1.astype(np.float64) @ Wl64 + bl.astype(np.float64))
    blpn = (bn.astype(np.float64) @ Wl64 + bl.astype(np.float64))

    def btile(blp):
        B = np.zeros((134,), np.float32)
        B[0:128] = blp.astype(np.float32)
        B[128:134] = (blp @ Cn).astype(np.float32)
        return np.tile(B[None, :], (128, 1)).copy()

    return rhsW(W1), rhsW(Wn), btile(blp1), btile(blpn), C1


def _table0(cfg, feats, C1, tdt):
    """Host-built layer-0 table [NTOT, 256] bf16 ((half, rank, local) rows)."""
    t = np.zeros((cfg.NTOT, cfg.ROW), np.float32)
    f64 = feats.astype(np.float64)
    elr = (f64 @ C1).astype(np.float32)
    rows = _row_of(cfg, np.arange(cfg.N))
    t[rows, 0:128] = feats
    t[rows, 128:134] = elr
    t[:, cfg.ONE] = 1.0
    return t.astype(tdt)


# ---------------------------------------------------------------------------
# device program
# ---------------------------------------------------------------------------

def build_program(cfg, nc_b, timing_mode=False, skip=()):
    import concourse.bass as bass
    import concourse.mybir as mybir
    import concourse.tile as tile
    from concourse import bacc

    dt = mybir.dt
    f32 = dt.float32
    bf16 = dt.bfloat16
    Alu = mybir.AluOpType
    Act = mybir.ActivationFunctionType

    NB, HB, SH, SHP = cfg.NB, cfg.HB, cfg.SH, cfg.SHP
    ROW, RHS = cfg.ROW, cfg.RHS
    TOT = int(nc_b.sum())
    NCMAX = int(nc_b.max())
    cum = np.concatenate([[0], np.cumsum(nc_b)]).astype(int)
    gpb = cfg.gpb
    groups = [list(range(gpb)), list(range(gpb, 2 * gpb))]

    nc = bacc.Bacc("TRN2", target_bir_lowering=False, debug=False,
                   num_devices=cfg.n_cores,
                   dynamic_dma_scratch_size=cfg.dma_scratch)

    # inputs -----------------------------------------------------------------
    t0_d = nc.dram_tensor("t0", [cfg.NTOT, ROW], bf16, kind="ExternalInput")
    rhs1_d = nc.dram_tensor("rhs1", [3, 128, 134], bf16, kind="ExternalInput")
    rhsn_d = nc.dram_tensor("rhsn", [3, 128, 134], bf16, kind="ExternalInput")
    bt1_d = nc.dram_tensor("bt1", [128, 134], f32, kind="ExternalInput")
    btn_d = nc.dram_tensor("btn", [128, 134], f32, kind="ExternalInput")
    iota_d = nc.dram_tensor("iota", [128, 128], bf16, kind="ExternalInput")
    ident_d = nc.dram_tensor("ident", [128, 128], f32, kind="ExternalInput")
    dst3_d = nc.dram_tensor("dst3", [128, TOT], f32, kind="ExternalInput")
    idx_d = nc.dram_tensor("idx", [128, TOT * 16], dt.int16,
                           kind="ExternalInput")
    poolw_d = nc.dram_tensor("poolw", [NB, 128, 128], bf16,
                             kind="ExternalInput")
    pool_out = nc.dram_tensor("pool_out", [128, 128], f32,
                              kind="ExternalOutput")

    # internal DRAM ----------------------------------------------------------
    ti_d = nc.dram_tensor("ti", [cfg.NTOT, ROW], bf16)  # layer-0 table copy
    t1_d = nc.dram_tensor("t1", [cfg.NTOT, ROW], bf16)
    t2_d = nc.dram_tensor("t2", [cfg.NTOT, ROW], bf16)
    HROW = HB * 128
    # local shard halves (standalone tensors; collective ins)
    tsh_d = [[nc.dram_tensor(f"tsh{l}{h}", [HROW, ROW], bf16)
              for h in range(2)] for l in range(2)]

    def do_gather(layer, tfull, half):
        """All-gather one half of the shard table into the full table."""
        tsh = tsh_d[layer][half]
        outs = tfull.ap()[half * gpb * HROW:(half + 1) * gpb * HROW,
                          :].rearrange("(q r) c -> q r c", q=gpb)
        if timing_mode:
            for j in range(gpb):
                nc.sync.dma_start(outs[j], tsh.ap())
        else:
            nc.gpsimd.collective_compute(
                "AllGather", mybir.AluOpType.bypass, replica_groups=groups,
                ins=[tsh.ap()], outs=[outs])

    with tile.TileContext(nc) as tc:
        cpool = tc.alloc_tile_pool(name="const", bufs=1)
        rhs1 = cpool.tile([128, 3, 134], bf16, tag="rhs1")
        rhsn = cpool.tile([128, 3, 134], bf16, tag="rhsn")
        bt1 = cpool.tile([128, 134], f32, tag="bt1")
        btn = cpool.tile([128, 134], f32, tag="btn")
        iota = cpool.tile([128, 128], bf16, tag="iota")
        identf = cpool.tile([128, 128], f32, tag="identf")
        dst3 = cpool.tile([128, TOT], f32, tag="dst3")
        idx = cpool.tile([128, TOT * 16], dt.int16, tag="idx")

        nc.sync.dma_start(rhs1[:], rhs1_d.ap().rearrange("k p m -> p k m"))
        nc.sync.dma_start(rhsn[:], rhsn_d.ap().rearrange("k p m -> p k m"))
        nc.sync.dma_start(bt1[:], bt1_d.ap())
        nc.sync.dma_start(btn[:], btn_d.ap())
        nc.sync.dma_start(iota[:], iota_d.ap())
        nc.sync.dma_start(identf[:], ident_d.ap())
        nc.sync.dma_start(dst3[:], dst3_d.ap())
        nc.sync.dma_start(idx[:], idx_d.ap())

        g_pool = tc.alloc_tile_pool(name="g", bufs=2)
        w_pool = tc.alloc_tile_pool(name="w", bufs=2)
        l_pool = tc.alloc_tile_pool(name="l", bufs=12)
        psb_pool = tc.alloc_tile_pool(name="psb", bufs=2, space="PSUM")
        s_pool = tc.alloc_tile_pool(name="s", bufs=2)
        u_pool = tc.alloc_tile_pool(name="u", bufs=6)
        pst_pool = tc.alloc_tile_pool(name="pst", bufs=2, space="PSUM")
        ut_pool = tc.alloc_tile_pool(name="ut", bufs=6)
        psx_pool = tc.alloc_tile_pool(name="psx", bufs=2, space="PSUM")
        x_pool = tc.alloc_tile_pool(name="x", bufs=3)
        pw_pool = tc.alloc_tile_pool(name="pw", bufs=2)
        pp_pool = tc.alloc_tile_pool(name="pp", bufs=1, space="PSUM")

        ps_pool_acc = pp_pool.tile([128, 128], f32, tag="poolacc")

        for layer in range(3):
            tbl = (t0_d, t1_d, t2_d)[layer]
            rw = rhs1 if layer == 0 else rhsn
            bt = bt1 if layer == 0 else btn
            for b in range(NB):
                ncb = int(nc_b[b])
                c0 = int(cum[b])
                # ---- merged gather: per chunk 128 src rows + 128 dst rows
                Gt = g_pool.tile([128, 2 * NCMAX, ROW], bf16, tag="G")
                if "gather" not in skip:
                    for g0 in range(0, ncb, cfg.GCH):
                        gsz = min(cfg.GCH, ncb - g0)
                        nc.gpsimd.dma_gather(
                            Gt[:, 2 * g0:2 * (g0 + gsz), :], tbl.ap(),
                            idx[:, 16 * (c0 + g0): 16 * (c0 + g0 + gsz)],
                            num_idxs=gsz * 256, num_idxs_reg=gsz * 256,
                            elem_size=ROW, elem_step=ROW)
                # ---- attention weights w = exp(leaky(el_src + er_dst))
                wt = w_pool.tile([128, NCMAX, 3], f32, tag="wt")
                nc.vector.tensor_tensor(
                    wt[:, 0:ncb, :], Gt[:, 0:2 * ncb:2, cfg.EL0:cfg.EL0 + 3],
                    Gt[:, 1:2 * ncb:2, cfg.ER0:cfg.ER0 + 3], Alu.add)
                nc.vector.scalar_tensor_tensor(
                    wt[:, 0:ncb, :], wt[:, 0:ncb, :], cfg.neg_slope,
                    wt[:, 0:ncb, :], Alu.mult, Alu.max)
                nc.scalar.activation(wt[:, 0:ncb, :], wt[:, 0:ncb, :],
                                     Act.Exp)
                # ---- scatter-add via w-scaled one-hot matmuls
                psb = psb_pool.tile([128, 3 * RHS], f32, tag="psb")
                for c in range(ncb):
                    cc = c0 + c
                    for h in range(3):
                        lh = l_pool.tile([128, 128], bf16, tag="lh")
                        if "onehot" not in skip:
                            nc.vector.tensor_scalar(
                                lh[:], iota[:], dst3[:, cc:cc + 1],
                                wt[:, c, h:h + 1].opt(),
                                Alu.is_equal, Alu.mult)
                        if "emm" in skip:
                            continue
                        nc.tensor.matmul(
                            psb[:, RHS * h:RHS * h + RHS], lh[:],
                            Gt[:, 2 * c, 0:RHS].opt(),
                            start=(c == 0 and h == 0),
                            stop=(c == ncb - 1 and h == 2))
                # ---- epilogue: normalize, transpose, apply M|MC
                r3 = s_pool.tile([128, 3], f32, tag="r3")
                nc.vector.reciprocal(r3[:], psb[:, RHS - 1::RHS])
                psx = psx_pool.tile([128, 134], f32, tag="psx")
                for h in range(3):
                    u = u_pool.tile([128, 128], f32, tag="u")
                    nc.scalar.activation(u[:], psb[:, RHS * h:RHS * h + 128],
                                         Act.Copy, scale=r3[:, h:h + 1])
                    pst = pst_pool.tile([128, 128], f32, tag="pst")
                    nc.tensor.transpose(pst[:], u[:], identf[:])
                    uT = ut_pool.tile([128, 128], bf16, tag="uT")
                    nc.scalar.activation(uT[:], pst[:], Act.Copy)
                    nc.tensor.matmul(psx[:], uT[:], rw[:, h, :].opt(),
                                     start=(h == 0), stop=(h == 2))
                xsb = x_pool.tile([128, ROW], bf16, tag="xsb")
                nc.vector.tensor_tensor(xsb[:, 0:134], psx[:], bt[:], Alu.add)
                if layer < 2:
                    nc.vector.memset(xsb[:, cfg.ONE:ROW], 1.0)
                    half, bh = (0, b) if b < HB else (1, b - HB)
                    nc.sync.dma_start(
                        tsh_d[layer][half].ap()[bh * 128:(bh + 1) * 128, :],
                        xsb[:])
                    if b == HB - 1:
                        do_gather(layer, (t1_d, t2_d)[layer], 0)
                    elif b == NB - 1:
                        do_gather(layer, (t1_d, t2_d)[layer], 1)
                else:
                    pw = pw_pool.tile([128, 128], bf16, tag="pw")
                    nc.sync.dma_start(pw[:], poolw_d.ap()[b])
                    nc.tensor.matmul(ps_pool_acc[:], pw[:], xsb[:, 0:128],
                                     start=(b == 0), stop=(b == NB - 1))

        po = x_pool.tile([128, 128], f32, tag="po")
        nc.vector.tensor_copy(po[:], ps_pool_acc[:])
        nc.sync.dma_start(pool_out.ap(), po[:])

        for p in (pp_pool, pw_pool, x_pool, psx_pool, ut_pool, pst_pool,
                  u_pool, s_pool, psb_pool, l_pool, w_pool, g_pool, cpool):
            p.release()

    nc.compile()
    return nc


# ---------------------------------------------------------------------------
# top-level kernel
# ---------------------------------------------------------------------------

def _prepare(cfg, inputs):
    """Returns (nc_b, in_maps, host_meta)."""
    npf = np.asarray
    import ml_dtypes
    tdt = ml_dtypes.bfloat16

    per_core_edges = []
    nc_b = np.zeros(cfg.NB, np.int64)
    for br, (s, d) in enumerate((("srcA", "dstA"), ("srcB", "dstB"))):
        src = npf(inputs[s]).astype(np.int64)
        dst = npf(inputs[d]).astype(np.int64)
        for q in range(cfg.gpb):
            es, ed, cnt = _prep_edges(cfg, src, dst, q)
            per_core_edges.append((es, ed))
            nc_b = np.maximum(nc_b, -(-cnt // 128))

    iota = np.tile(np.arange(128, dtype=tdt), (128, 1))
    ident = np.eye(128, dtype=np.float32)

    in_maps = []
    host_meta = {}
    for br in range(2):
        sfx = "AB"[br]
        W1 = npf(inputs["W1" + sfx]); al1 = npf(inputs["al1" + sfx])
        ar1 = npf(inputs["ar1" + sfx]); b1 = npf(inputs["b1" + sfx])
        Wn = npf(inputs["Wn" + sfx]); aln = npf(inputs["aln" + sfx])
        arn = npf(inputs["arn" + sfx]); bn = npf(inputs["bn" + sfx])
        Wl = npf(inputs["Wl" + sfx]); bl = npf(inputs["bl" + sfx])
        gid = npf(inputs["gid" + sfx]).astype(np.int64)
        feats = npf(inputs["feats" + sfx]).astype(np.float32)
        rhs1, rhsn, bt1, btn, C1 = _branch_consts(
            cfg, W1, al1, ar1, b1, Wn, aln, arn, bn, Wl, bl)
        t0 = _table0(cfg, feats, C1, tdt)
        host_meta[sfx] = dict(gid=gid)
        for q in range(cfg.gpb):
            es, ed = per_core_edges[br * cfg.gpb + q]
            idx, d3 = _pack_core(cfg, es, ed, q, nc_b)
            poolw = np.zeros((cfg.NB, 128, 128), tdt)
            for b in range(cfg.NB):
                for i in range(min(128, cfg.SH - b * 128)):
                    n = q * cfg.SH + b * 128 + i
                    if n < cfg.N:
                        poolw[b, i, gid[n]] = 1.0
            in_maps.append({
                "t0": t0,
                "rhs1": rhs1.astype(tdt), "rhsn": rhsn.astype(tdt),
                "bt1": bt1, "btn": btn,
                "iota": iota, "ident": ident,
                "dst3": d3, "idx": idx, "poolw": poolw,
            })
    return nc_b, in_maps, host_meta


def _finalize(cfg, inputs, host_meta, pool_outs):
    """pool_outs: list of 8 [128,128] arrays -> full output [G,1] float64."""
    out = {}
    for br in range(2):
        sfx = "AB"[br]
        total = np.zeros((128, 128), np.float64)
        for q in range(cfg.gpb):
            total += pool_outs[br * cfg.gpb + q].astype(np.float64)
        gid = host_meta[sfx]["gid"]
        cnt = np.bincount(gid, minlength=128).astype(np.float64)
        out[sfx] = (total / np.maximum(cnt[:, None], 1.0))[:cfg.G]
    cat = np.concatenate([out["A"], out["B"]], axis=1)
    Wo = np.asarray(inputs["Wo"]).astype(np.float64)
    bo = np.asarray(inputs["bo"]).astype(np.float64)
    return (cat @ Wo + bo).astype(np.float64)


_CACHE = {}


def kernel(**inputs):
    cfg = Cfg(N=inputs["featsA"].shape[0], G=128)
    nc_b, in_maps, host_meta = _prepare(cfg, inputs)
    key = ("prog", tuple(nc_b.tolist()))
    if key not in _CACHE:
        _CACHE[key] = build_program(cfg, nc_b)
    nc = _CACHE[key]
    from concourse.bass_utils import run_bass_kernel_spmd
    res = run_bass_kernel_spmd(nc, in_maps, list(range(cfg.n_cores)))
    pool_outs = [r["pool_out"] for r in res.results]
    return _finalize(cfg, inputs, host_meta, pool_outs)


# revision 47
# speedup vs baseline: 1.7111x; 1.1634x over previous
"""Trainium2 Bass kernel for nn_DoubleNet (two GATNet branches + avg-pool + linear).

Strategy (8 NeuronCores), "x-gather" design:
  - Cores 0-3 run branch A, cores 4-7 run branch B (same SPMD program,
    different data). Within a branch, dst nodes are sharded across 4 cores.
  - Key algebraic refactor: for one GAT layer followed by the shared linear,
      x_next[d] = sum_h (sum_e a_e^h x[src_e]) @ M_h + blp,   M_h = W_h @ Wl_h
    so the edge phase only needs x[src] (128 cols) per edge instead of
    z[src] (384 cols), and the per-head linear maps are applied AFTER
    aggregation, per 128-dst block. el/er attention scalars are linear in x
    too (el = x @ (W_h al_h)), kept in the per-node table row.
  - Per-layer node table (DRAM, bf16 rows of 512B):
      row(n) = [x (128) | el (3) | er (3) | 1 | pad] ; row id = q*5120 + r.
    Layer-0 table is host-built from feats; later tables are written
    per-block by the epilogue and all-gathered across the branch's 4 cores
    in two halves for overlap.
  - Edge phase per dst block: ONE merged dma_gather call pulls, per 128-edge
    chunk, 128 src rows and 128 dst rows (interleaved slabs; er comes from
    the dst rows). w = exp(leaky(el+er)); per head a w-scaled one-hot matmul
    scatter-adds messages+denominator into PSUM ([x|...|1] rhs, 135 cols).
  - Epilogue per block: normalize per head (Act copy w/ scale), PE-transpose,
    3 matmuls against [M_h | M_h@Cnext] (134 cols) -> x_next and next-layer
    el/er in one shot; bias-add; row-write to the next table (layers 0,1) or
    one-hot pool matmul (layer 2). Host divides pooled sums by graph counts
    and applies the output linear.
"""

import sys

sys.path.insert(0, "/opt/trn_rl_repo")

import numpy as np


# ---------------------------------------------------------------------------
# configuration
# ---------------------------------------------------------------------------

class Cfg:
    def __init__(self, N=20000, G=128, H=3, EMB=128, F=128, n_cores=8,
                 neg_slope=0.2):
        assert F == 128 and EMB == 128 and H == 3
        self.N, self.G, self.H, self.EMB, self.F = N, G, H, EMB, F
        self.n_cores = n_cores
        self.gpb = n_cores // 2            # cores per branch
        assert N % self.gpb == 0
        self.SH = N // self.gpb            # dst nodes per core
        self.NB = -(-self.SH // 128)       # dst blocks per core
        self.SHP = self.NB * 128           # padded rows per shard
        self.NTOT = self.gpb * self.SHP    # table rows
        self.HB = self.NB // 2
        self.neg_slope = neg_slope
        # table row layout (bf16 cols): [x 0:128 | el 128:131 | er 131:134 |
        #   one @134 | pad to 256]. er window = cols 128:256 of the row.
        self.ROW = 256
        self.EL0, self.ER0, self.ONE = 128, 131, 134
        self.RHS = 135                     # matmul rhs cols [x|el|er|1]
        self.GCH = 4                       # chunks per gather call (256 idx each)
        self.dma_scratch = 32768           # SWDGE ring size driver


# ---------------------------------------------------------------------------
# host-side data prep
# ---------------------------------------------------------------------------

def _prep_edges(cfg, src, dst, q):
    """Edges of one core (dst in its shard), dst-sorted, fake rows added."""
    lo = q * cfg.SH
    sel = (dst >= lo) & (dst < lo + cfg.SH)
    es = src[sel].astype(np.int64)
    ed = (dst[sel].astype(np.int64) - lo)
    nfake = cfg.SHP - cfg.SH
    if nfake:
        es = np.concatenate([es, np.zeros(nfake, np.int64)])
        ed = np.concatenate([ed, np.arange(cfg.SH, cfg.SHP, dtype=np.int64)])
    order = np.argsort(ed, kind="stable")
    es, ed = es[order], ed[order]
    cnt = np.bincount(ed // 128, minlength=cfg.NB)
    return es, ed, cnt


def _row_of(cfg, n):
    """Global node id -> table row id.

    Rows are grouped (half, rank, local) with HROW = HB*128 rows per rank per
    half, so each all-gather half lands in one contiguous table region.
    """
    q, r = n // cfg.SH, n % cfg.SH
    HROW = cfg.HB * 128
    return np.where(r < HROW, q * HROW + r,
                    cfg.gpb * HROW + q * HROW + (r - HROW))


def _pack_core(cfg, es, ed, q, nc_b):
    """Merged (src,dst) interleaved index array + dst3 per chunk."""
    TOT = int(nc_b.sum())
    idx = np.zeros(TOT * 256, np.int16)    # per chunk: 128 src rows, 128 dst
    dst3 = np.full(TOT * 128, -1.0, np.float32)
    epos = np.searchsorted(ed, np.arange(0, cfg.SHP + 1, 128))
    c0 = 0
    for b in range(cfg.NB):
        s, e = epos[b], epos[b + 1]
        srows = _row_of(cfg, es[s:e])
        HROW = cfg.HB * 128
        dl = ed[s:e]
        drows = np.where(dl < HROW, q * HROW + dl,
                         cfg.gpb * HROW + q * HROW + (dl - HROW))
        off = ed[s:e] - b * 128
        for c in range(int(nc_b[b])):
            o, n = c * 128, min(128, (e - s) - c * 128)
            if n <= 0:
                break
            cc = c0 + c
            idx[cc * 256: cc * 256 + n] = srows[o:o + n]
            idx[cc * 256 + 128: cc * 256 + 128 + n] = drows[o:o + n]
            dst3[cc * 128: cc * 128 + n] = off[o:o + n].astype(np.float32)
        c0 += int(nc_b[b])

    def wrap(a):  # flat i -> (partition i%16, col i//16), replicated to 128
        return np.tile(a.reshape(-1, 16).T, (8, 1)).copy()

    d3 = dst3.reshape(TOT, 128).T.copy()
    return wrap(idx), d3


def _branch_consts(cfg, W1, al1, ar1, b1, Wn, aln, arn, bn, Wl, bl):
    """Per-branch folded weights: M|MC rhs tensors, bias tiles, el/er coeffs."""
    H, EMB = cfg.H, cfg.EMB

    def coeffs(W, al, ar):
        C = np.zeros((W.shape[0], 6), np.float64)
        for h in range(H):
            Wh = W[:, h * EMB:(h + 1) * EMB].astype(np.float64)
            C[:, h] = Wh @ al[h].astype(np.float64)
            C[:, 3 + h] = Wh @ ar[h].astype(np.float64)
        return C

    Wl64 = Wl.astype(np.float64)
    C1 = coeffs(W1, al1, ar1)
    Cn = coeffs(Wn, aln, arn)

    def rhsW(W):
        R = np.zeros((H, W.shape[0], 134), np.float32)
        for h in range(H):
            Wh = W[:, h * EMB:(h + 1) * EMB].astype(np.float64)
            M = Wh @ Wl64[h * EMB:(h + 1) * EMB, :]
            R[h, :, 0:128] = M.astype(np.float32)
            R[h, :, 128:134] = (M @ Cn).astype(np.float32)
        return R

    blp1 = (b1.astype(np.float64) @ Wl64 + bl.astype(np.float64))
    blpn = (bn.astype(np.float64) @ Wl64 + bl.astype(np.float64))

    def btile(blp):
        B = np.zeros((134,), np.float32)
        B[0:128] = blp.astype(np.float32)
        B[128:134] = (blp @ Cn).astype(np.float32)
        return np.tile(B[None, :], (128, 1)).copy()

    return rhsW(W1), rhsW(Wn), btile(blp1), btile(blpn), C1


def _table0(cfg, feats, C1, tdt):
    """Host-built layer-0 table [NTOT, 256] bf16 ((half, rank, local) rows)."""
    t = np.zeros((cfg.NTOT, cfg.ROW), np.float32)
    f64 = feats.astype(np.float64)
    elr = (f64 @ C1).astype(np.float32)
    rows = _row_of(cfg, np.arange(cfg.N))
    t[rows, 0:128] = feats
    t[rows, 128:134] = elr
    t[:, cfg.ONE] = 1.0
    return t.astype(tdt)


# ---------------------------------------------------------------------------
# device program
# ---------------------------------------------------------------------------

def build_program(cfg, nc_b, timing_mode=False, skip=()):
    import concourse.bass as bass
    import concourse.mybir as mybir
    import concourse.tile as tile
    from concourse import bacc

    dt = mybir.dt
    f32 = dt.float32
    bf16 = dt.bfloat16
    Alu = mybir.AluOpType
    Act = mybir.ActivationFunctionType

    NB, HB, SH, SHP = cfg.NB, cfg.HB, cfg.SH, cfg.SHP
    ROW, RHS = cfg.ROW, cfg.RHS
    TOT = int(nc_b.sum())
    NCMAX = int(nc_b.max())
    cum = np.concatenate([[0], np.cumsum(nc_b)]).astype(int)
    gpb = cfg.gpb
    groups = [list(range(gpb)), list(range(gpb, 2 * gpb))]

    nc = bacc.Bacc("TRN2", target_bir_lowering=False, debug=False,
                   num_devices=cfg.n_cores,
                   dynamic_dma_scratch_size=cfg.dma_scratch)

    # inputs -----------------------------------------------------------------
    t0_d = nc.dram_tensor("t0", [cfg.NTOT, ROW], bf16, kind="ExternalInput")
    rhs1_d = nc.dram_tensor("rhs1", [3, 128, 134], bf16, kind="ExternalInput")
    rhsn_d = nc.dram_tensor("rhsn", [3, 128, 134], bf16, kind="ExternalInput")
    bt1_d = nc.dram_tensor("bt1", [128, 134], f32, kind="ExternalInput")
    btn_d = nc.dram_tensor("btn", [128, 134], f32, kind="ExternalInput")
    iota_d = nc.dram_tensor("iota", [128, 128], bf16, kind="ExternalInput")
    ident_d = nc.dram_tensor("ident", [128, 128], f32, kind="ExternalInput")
    dst3_d = nc.dram_tensor("dst3", [128, TOT], f32, kind="ExternalInput")
    idx_d = nc.dram_tensor("idx", [128, TOT * 16], dt.int16,
                           kind="ExternalInput")
    poolw_d = nc.dram_tensor("poolw", [NB, 128, 128], bf16,
                             kind="ExternalInput")
    pool_out = nc.dram_tensor("pool_out", [128, 128], f32,
                              kind="ExternalOutput")

    # internal DRAM ----------------------------------------------------------
    ti_d = nc.dram_tensor("ti", [cfg.NTOT, ROW], bf16)  # layer-0 table copy
    t1_d = nc.dram_tensor("t1", [cfg.NTOT, ROW], bf16)
    t2_d = nc.dram_tensor("t2", [cfg.NTOT, ROW], bf16)
    HROW = HB * 128
    # local shard halves (standalone tensors; collective ins)
    tsh_d = [[nc.dram_tensor(f"tsh{l}{h}", [HROW, ROW], bf16)
              for h in range(2)] for l in range(2)]

    def do_gather(layer, tfull, half):
        """All-gather one half of the shard table into the full table."""
        tsh = tsh_d[layer][half]
        outs = tfull.ap()[half * gpb * HROW:(half + 1) * gpb * HROW,
                          :].rearrange("(q r) c -> q r c", q=gpb)
        if timing_mode:
            for j in range(gpb):
                nc.sync.dma_start(outs[j], tsh.ap())
        else:
            nc.gpsimd.collective_compute(
                "AllGather", mybir.AluOpType.bypass, replica_groups=groups,
                ins=[tsh.ap()], outs=[outs])

    with tile.TileContext(nc) as tc:
        cpool = tc.alloc_tile_pool(name="const", bufs=1)
        rhs1 = cpool.tile([128, 3, 134], bf16, tag="rhs1")
        rhsn = cpool.tile([128, 3, 134], bf16, tag="rhsn")
        bt1 = cpool.tile([128, 134], f32, tag="bt1")
        btn = cpool.tile([128, 134], f32, tag="btn")
        iota = cpool.tile([128, 128], bf16, tag="iota")
        identf = cpool.tile([128, 128], f32, tag="identf")
        dst3 = cpool.tile([128, TOT], f32, tag="dst3")
        idx = cpool.tile([128, TOT * 16], dt.int16, tag="idx")

        nc.sync.dma_start(rhs1[:], rhs1_d.ap().rearrange("k p m -> p k m"))
        nc.sync.dma_start(rhsn[:], rhsn_d.ap().rearrange("k p m -> p k m"))
        nc.sync.dma_start(bt1[:], bt1_d.ap())
        nc.sync.dma_start(btn[:], btn_d.ap())
        nc.sync.dma_start(iota[:], iota_d.ap())
        nc.sync.dma_start(identf[:], ident_d.ap())
        nc.sync.dma_start(dst3[:], dst3_d.ap())
        nc.sync.dma_start(idx[:], idx_d.ap())

        g_pool = tc.alloc_tile_pool(name="g", bufs=4)
        w_pool = tc.alloc_tile_pool(name="w", bufs=3)
        l_pool = tc.alloc_tile_pool(name="l", bufs=18)
        psb_pool = tc.alloc_tile_pool(name="psb", bufs=2, space="PSUM")
        s_pool = tc.alloc_tile_pool(name="s", bufs=2)
        u_pool = tc.alloc_tile_pool(name="u", bufs=6)
        pst_pool = tc.alloc_tile_pool(name="pst", bufs=2, space="PSUM")
        ut_pool = tc.alloc_tile_pool(name="ut", bufs=6)
        psx_pool = tc.alloc_tile_pool(name="psx", bufs=2, space="PSUM")
        x_pool = tc.alloc_tile_pool(name="x", bufs=3)
        pw_pool = tc.alloc_tile_pool(name="pw", bufs=2)
        pp_pool = tc.alloc_tile_pool(name="pp", bufs=1, space="PSUM")

        ps_pool_acc = pp_pool.tile([128, 128], f32, tag="poolacc")

        for layer in range(3):
            tbl = (t0_d, t1_d, t2_d)[layer]
            rw = rhs1 if layer == 0 else rhsn
            bt = bt1 if layer == 0 else btn
            for b in range(NB):
                ncb = int(nc_b[b])
                c0 = int(cum[b])
                # ---- merged gather: per chunk 128 src rows + 128 dst rows
                Gt = g_pool.tile([128, 2 * NCMAX, ROW], bf16, tag="G")
                wt = w_pool.tile([128, NCMAX, 3], f32, tag="wt")
                for g0 in range(0, ncb, cfg.GCH):
                    gsz = min(cfg.GCH, ncb - g0)
                    if "gather" not in skip:
                        nc.gpsimd.dma_gather(
                            Gt[:, 2 * g0:2 * (g0 + gsz), :], tbl.ap(),
                            idx[:, 16 * (c0 + g0): 16 * (c0 + g0 + gsz)],
                            num_idxs=gsz * 256, num_idxs_reg=gsz * 256,
                            elem_size=ROW, elem_step=ROW)
                    # ---- attention weights w = exp(leaky(el_src + er_dst))
                    sl = slice(g0, g0 + gsz)
                    nc.vector.tensor_tensor(
                        wt[:, sl, :],
                        Gt[:, 2 * g0:2 * (g0 + gsz):2, cfg.EL0:cfg.EL0 + 3],
                        Gt[:, 2 * g0 + 1:2 * (g0 + gsz):2,
                           cfg.ER0:cfg.ER0 + 3], Alu.add)
                    nc.vector.scalar_tensor_tensor(
                        wt[:, sl, :], wt[:, sl, :], cfg.neg_slope,
                        wt[:, sl, :], Alu.mult, Alu.max)
                    nc.scalar.activation(wt[:, sl, :], wt[:, sl, :], Act.Exp)
                # ---- scatter-add via w-scaled one-hot matmuls
                psb = psb_pool.tile([128, 3 * RHS], f32, tag="psb")
                for c in range(ncb):
                    cc = c0 + c
                    for h in range(3):
                        lh = l_pool.tile([128, 128], bf16, tag="lh")
                        if "onehot" not in skip:
                            nc.vector.tensor_scalar(
                                lh[:], iota[:], dst3[:, cc:cc + 1],
                                wt[:, c, h:h + 1].opt(),
                                Alu.is_equal, Alu.mult)
                        if "emm" in skip:
                            continue
                        nc.tensor.matmul(
                            psb[:, RHS * h:RHS * h + RHS], lh[:],
                            Gt[:, 2 * c, 0:RHS].opt(),
                            start=(c == 0 and h == 0),
                            stop=(c == ncb - 1 and h == 2))
                # ---- epilogue: normalize, transpose, apply M|MC
                r3 = s_pool.tile([128, 3], f32, tag="r3")
                nc.vector.reciprocal(r3[:], psb[:, RHS - 1::RHS])
                psx = psx_pool.tile([128, 134], f32, tag="psx")
                for h in range(3):
                    u = u_pool.tile([128, 128], f32, tag="u")
                    nc.scalar.activation(u[:], psb[:, RHS * h:RHS * h + 128],
                                         Act.Copy, scale=r3[:, h:h + 1])
                    pst = pst_pool.tile([128, 128], f32, tag="pst")
                    nc.tensor.transpose(pst[:], u[:], identf[:])
                    uT = ut_pool.tile([128, 128], bf16, tag="uT")
                    nc.scalar.activation(uT[:], pst[:], Act.Copy)
                    nc.tensor.matmul(psx[:], uT[:], rw[:, h, :].opt(),
                                     start=(h == 0), stop=(h == 2))
                xsb = x_pool.tile([128, ROW], bf16, tag="xsb")
                nc.vector.tensor_tensor(xsb[:, 0:134], psx[:], bt[:], Alu.add)
                if layer < 2:
                    nc.vector.memset(xsb[:, cfg.ONE:ROW], 1.0)
                    half, bh = (0, b) if b < HB else (1, b - HB)
                    nc.sync.dma_start(
                        tsh_d[layer][half].ap()[bh * 128:(bh + 1) * 128, :],
                        xsb[:])
                    if b == HB - 1:
                        do_gather(layer, (t1_d, t2_d)[layer], 0)
                    elif b == NB - 1:
                        do_gather(layer, (t1_d, t2_d)[layer], 1)
                else:
                    pw = pw_pool.tile([128, 128], bf16, tag="pw")
                    nc.sync.dma_start(pw[:], poolw_d.ap()[b])
                    nc.tensor.matmul(ps_pool_acc[:], pw[:], xsb[:, 0:128],
                                     start=(b == 0), stop=(b == NB - 1))

        po = x_pool.tile([128, 128], f32, tag="po")
        nc.vector.tensor_copy(po[:], ps_pool_acc[:])
        nc.sync.dma_start(pool_out.ap(), po[:])

        for p in (pp_pool, pw_pool, x_pool, psx_pool, ut_pool, pst_pool,
                  u_pool, s_pool, psb_pool, l_pool, w_pool, g_pool, cpool):
            p.release()

    nc.compile()
    return nc


# ---------------------------------------------------------------------------
# top-level kernel
# ---------------------------------------------------------------------------

def _prepare(cfg, inputs):
    """Returns (nc_b, in_maps, host_meta)."""
    npf = np.asarray
    import ml_dtypes
    tdt = ml_dtypes.bfloat16

    per_core_edges = []
    nc_b = np.zeros(cfg.NB, np.int64)
    for br, (s, d) in enumerate((("srcA", "dstA"), ("srcB", "dstB"))):
        src = npf(inputs[s]).astype(np.int64)
        dst = npf(inputs[d]).astype(np.int64)
        for q in range(cfg.gpb):
            es, ed, cnt = _prep_edges(cfg, src, dst, q)
            per_core_edges.append((es, ed))
            nc_b = np.maximum(nc_b, -(-cnt // 128))

    iota = np.tile(np.arange(128, dtype=tdt), (128, 1))
    ident = np.eye(128, dtype=np.float32)

    in_maps = []
    host_meta = {}
    for br in range(2):
        sfx = "AB"[br]
        W1 = npf(inputs["W1" + sfx]); al1 = npf(inputs["al1" + sfx])
        ar1 = npf(inputs["ar1" + sfx]); b1 = npf(inputs["b1" + sfx])
        Wn = npf(inputs["Wn" + sfx]); aln = npf(inputs["aln" + sfx])
        arn = npf(inputs["arn" + sfx]); bn = npf(inputs["bn" + sfx])
        Wl = npf(inputs["Wl" + sfx]); bl = npf(inputs["bl" + sfx])
        gid = npf(inputs["gid" + sfx]).astype(np.int64)
        feats = npf(inputs["feats" + sfx]).astype(np.float32)
        rhs1, rhsn, bt1, btn, C1 = _branch_consts(
            cfg, W1, al1, ar1, b1, Wn, aln, arn, bn, Wl, bl)
        t0 = _table0(cfg, feats, C1, tdt)
        host_meta[sfx] = dict(gid=gid)
        for q in range(cfg.gpb):
            es, ed = per_core_edges[br * cfg.gpb + q]
            idx, d3 = _pack_core(cfg, es, ed, q, nc_b)
            poolw = np.zeros((cfg.NB, 128, 128), tdt)
            for b in range(cfg.NB):
                for i in range(min(128, cfg.SH - b * 128)):
                    n = q * cfg.SH + b * 128 + i
                    if n < cfg.N:
                        poolw[b, i, gid[n]] = 1.0
            in_maps.append({
                "t0": t0,
                "rhs1": rhs1.astype(tdt), "rhsn": rhsn.astype(tdt),
                "bt1": bt1, "btn": btn,
                "iota": iota, "ident": ident,
                "dst3": d3, "idx": idx, "poolw": poolw,
            })
    return nc_b, in_maps, host_meta


def _finalize(cfg, inputs, host_meta, pool_outs):
    """pool_outs: list of 8 [128,128] arrays -> full output [G,1] float64."""
    out = {}
    for br in range(2):
        sfx = "AB"[br]
        total = np.zeros((128, 128), np.float64)
        for q in range(cfg.gpb):
            total += pool_outs[br * cfg.gpb + q].astype(np.float64)
        gid = host_meta[sfx]["gid"]
        cnt = np.bincount(gid, minlength=128).astype(np.float64)
        out[sfx] = (total / np.maximum(cnt[:, None], 1.0))[:cfg.G]
    cat = np.concatenate([out["A"], out["B"]], axis=1)
    Wo = np.asarray(inputs["Wo"]).astype(np.float64)
    bo = np.asarray(inputs["bo"]).astype(np.float64)
    return (cat @ Wo + bo).astype(np.float64)


_CACHE = {}


def kernel(**inputs):
    cfg = Cfg(N=inputs["featsA"].shape[0], G=128)
    nc_b, in_maps, host_meta = _prepare(cfg, inputs)
    key = ("prog", tuple(nc_b.tolist()))
    if key not in _CACHE:
        _CACHE[key] = build_program(cfg, nc_b)
    nc = _CACHE[key]
    from concourse.bass_utils import run_bass_kernel_spmd
    res = run_bass_kernel_spmd(nc, in_maps, list(range(cfg.n_cores)))
    pool_outs = [r["pool_out"] for r in res.results]
    return _finalize(cfg, inputs, host_meta, pool_outs)


# revision 49
# speedup vs baseline: 1.7123x; 1.0007x over previous
"""Trainium2 Bass kernel for nn_DoubleNet (two GATNet branches + avg-pool + linear).

Strategy (8 NeuronCores), "x-gather" design:
  - Cores 0-3 run branch A, cores 4-7 run branch B (same SPMD program,
    different data). Within a branch, dst nodes are sharded across 4 cores.
  - Key algebraic refactor: for one GAT layer followed by the shared linear,
      x_next[d] = sum_h (sum_e a_e^h x[src_e]) @ M_h + blp,   M_h = W_h @ Wl_h
    so the edge phase only needs x[src] (128 cols) per edge instead of
    z[src] (384 cols), and the per-head linear maps are applied AFTER
    aggregation, per 128-dst block. el/er attention scalars are linear in x
    too (el = x @ (W_h al_h)), kept in the per-node table row.
  - Per-layer node table (DRAM, bf16 rows of 512B):
      row(n) = [x (128) | el (3) | er (3) | 1 | pad] ; row id = q*5120 + r.
    Layer-0 table is host-built from feats; later tables are written
    per-block by the epilogue and all-gathered across the branch's 4 cores
    in two halves for overlap.
  - Edge phase per dst block: ONE merged dma_gather call pulls, per 128-edge
    chunk, 128 src rows and 128 dst rows (interleaved slabs; er comes from
    the dst rows). w = exp(leaky(el+er)); per head a w-scaled one-hot matmul
    scatter-adds messages+denominator into PSUM ([x|...|1] rhs, 135 cols).
  - Epilogue per block: normalize per head (Act copy w/ scale), PE-transpose,
    3 matmuls against [M_h | M_h@Cnext] (134 cols) -> x_next and next-layer
    el/er in one shot; bias-add; row-write to the next table (layers 0,1) or
    one-hot pool matmul (layer 2). Host divides pooled sums by graph counts
    and applies the output linear.
"""

import sys

sys.path.insert(0, "/opt/trn_rl_repo")

import numpy as np


# ---------------------------------------------------------------------------
# configuration
# ---------------------------------------------------------------------------

class Cfg:
    def __init__(self, N=20000, G=128, H=3, EMB=128, F=128, n_cores=8,
                 neg_slope=0.2):
        assert F == 128 and EMB == 128 and H == 3
        self.N, self.G, self.H, self.EMB, self.F = N, G, H, EMB, F
        self.n_cores = n_cores
        self.gpb = n_cores // 2            # cores per branch
        assert N % self.gpb == 0
        self.SH = N // self.gpb            # dst nodes per core
        self.NB = -(-self.SH // 128)       # dst blocks per core
        self.SHP = self.NB * 128           # padded rows per shard
        self.NTOT = self.gpb * self.SHP    # table rows
        self.HB = self.NB // 2
        self.neg_slope = neg_slope
        # table row layout (bf16 cols): [x 0:128 | el 128:131 | er 131:134 |
        #   one @134 | pad to 256]. er window = cols 128:256 of the row.
        self.ROW = 256
        self.EL0, self.ER0, self.ONE = 128, 131, 134
        self.RHS = 135                     # matmul rhs cols [x|el|er|1]
        self.GCH = 4                       # chunks per gather call (256 idx each)
        self.dma_scratch = 32768           # SWDGE ring size driver


# ---------------------------------------------------------------------------
# host-side data prep
# ---------------------------------------------------------------------------

def _prep_edges(cfg, src, dst, q):
    """Edges of one core (dst in its shard), dst-sorted, fake rows added."""
    lo = q * cfg.SH
    sel = (dst >= lo) & (dst < lo + cfg.SH)
    es = src[sel].astype(np.int64)
    ed = (dst[sel].astype(np.int64) - lo)
    nfake = cfg.SHP - cfg.SH
    if nfake:
        es = np.concatenate([es, np.zeros(nfake, np.int64)])
        ed = np.concatenate([ed, np.arange(cfg.SH, cfg.SHP, dtype=np.int64)])
    order = np.argsort(ed, kind="stable")
    es, ed = es[order], ed[order]
    cnt = np.bincount(ed // 128, minlength=cfg.NB)
    return es, ed, cnt


def _row_of(cfg, n):
    """Global node id -> table row id.

    Rows are grouped (half, rank, local) with HROW = HB*128 rows per rank per
    half, so each all-gather half lands in one contiguous table region.
    """
    q, r = n // cfg.SH, n % cfg.SH
    HROW = cfg.HB * 128
    return np.where(r < HROW, q * HROW + r,
                    cfg.gpb * HROW + q * HROW + (r - HROW))


def _pack_core(cfg, es, ed, q, nc_b):
    """Merged (src,dst) interleaved index array + dst3 per chunk."""
    TOT = int(nc_b.sum())
    idx = np.zeros(TOT * 256, np.int16)    # per chunk: 128 src rows, 128 dst
    dst3 = np.full(TOT * 128, -1.0, np.float32)
    epos = np.searchsorted(ed, np.arange(0, cfg.SHP + 1, 128))
    c0 = 0
    for b in range(cfg.NB):
        s, e = epos[b], epos[b + 1]
        srows = _row_of(cfg, es[s:e])
        HROW = cfg.HB * 128
        dl = ed[s:e]
        drows = np.where(dl < HROW, q * HROW + dl,
                         cfg.gpb * HROW + q * HROW + (dl - HROW))
        off = ed[s:e] - b * 128
        for c in range(int(nc_b[b])):
            o, n = c * 128, min(128, (e - s) - c * 128)
            if n <= 0:
                break
            cc = c0 + c
            idx[cc * 256: cc * 256 + n] = srows[o:o + n]
            idx[cc * 256 + 128: cc * 256 + 128 + n] = drows[o:o + n]
            dst3[cc * 128: cc * 128 + n] = off[o:o + n].astype(np.float32)
        c0 += int(nc_b[b])

    def wrap(a):  # flat i -> (partition i%16, col i//16), replicated to 128
        return np.tile(a.reshape(-1, 16).T, (8, 1)).copy()

    d3 = dst3.reshape(TOT, 128).T.copy()
    return wrap(idx), d3


def _branch_consts(cfg, W1, al1, ar1, b1, Wn, aln, arn, bn, Wl, bl):
    """Per-branch folded weights: M|MC rhs tensors, bias tiles, el/er coeffs."""
    H, EMB = cfg.H, cfg.EMB

    def coeffs(W, al, ar):
        C = np.zeros((W.shape[0], 6), np.float64)
        for h in range(H):
            Wh = W[:, h * EMB:(h + 1) * EMB].astype(np.float64)
            C[:, h] = Wh @ al[h].astype(np.float64)
            C[:, 3 + h] = Wh @ ar[h].astype(np.float64)
        return C

    Wl64 = Wl.astype(np.float64)
    C1 = coeffs(W1, al1, ar1)
    Cn = coeffs(Wn, aln, arn)

    def rhsW(W):
        R = np.zeros((H, W.shape[0], 134), np.float32)
        for h in range(H):
            Wh = W[:, h * EMB:(h + 1) * EMB].astype(np.float64)
            M = Wh @ Wl64[h * EMB:(h + 1) * EMB, :]
            R[h, :, 0:128] = M.astype(np.float32)
            R[h, :, 128:134] = (M @ Cn).astype(np.float32)
        return R

    blp1 = (b1.astype(np.float64) @ Wl64 + bl.astype(np.float64))
    blpn = (bn.astype(np.float64) @ Wl64 + bl.astype(np.float64))

    def btile(blp):
        B = np.zeros((134,), np.float32)
        B[0:128] = blp.astype(np.float32)
        B[128:134] = (blp @ Cn).astype(np.float32)
        return np.tile(B[None, :], (128, 1)).copy()

    return rhsW(W1), rhsW(Wn), btile(blp1), btile(blpn), C1


def _table0(cfg, feats, C1, tdt):
    """Host-built layer-0 table [NTOT, 256] bf16 ((half, rank, local) rows)."""
    t = np.zeros((cfg.NTOT, cfg.ROW), np.float32)
    f64 = feats.astype(np.float64)
    elr = (f64 @ C1).astype(np.float32)
    rows = _row_of(cfg, np.arange(cfg.N))
    t[rows, 0:128] = feats
    t[rows, 128:134] = elr
    t[:, cfg.ONE] = 1.0
    return t.astype(tdt)


# ---------------------------------------------------------------------------
# device program
# ---------------------------------------------------------------------------

def build_program(cfg, nc_b, timing_mode=False, skip=()):
    import concourse.bass as bass
    import concourse.mybir as mybir
    import concourse.tile as tile
    from concourse import bacc

    dt = mybir.dt
    f32 = dt.float32
    bf16 = dt.bfloat16
    Alu = mybir.AluOpType
    Act = mybir.ActivationFunctionType

    NB, HB, SH, SHP = cfg.NB, cfg.HB, cfg.SH, cfg.SHP
    ROW, RHS = cfg.ROW, cfg.RHS
    TOT = int(nc_b.sum())
    NCMAX = int(nc_b.max())
    cum = np.concatenate([[0], np.cumsum(nc_b)]).astype(int)
    gpb = cfg.gpb
    groups = [list(range(gpb)), list(range(gpb, 2 * gpb))]

    nc = bacc.Bacc("TRN2", target_bir_lowering=False, debug=False,
                   num_devices=cfg.n_cores,
                   dynamic_dma_scratch_size=cfg.dma_scratch)

    # inputs -----------------------------------------------------------------
    t0_d = nc.dram_tensor("t0", [cfg.NTOT, ROW], bf16, kind="ExternalInput")
    rhs1_d = nc.dram_tensor("rhs1", [3, 128, 134], bf16, kind="ExternalInput")
    rhsn_d = nc.dram_tensor("rhsn", [3, 128, 134], bf16, kind="ExternalInput")
    bt1_d = nc.dram_tensor("bt1", [128, 134], f32, kind="ExternalInput")
    btn_d = nc.dram_tensor("btn", [128, 134], f32, kind="ExternalInput")
    iota_d = nc.dram_tensor("iota", [128, 128], bf16, kind="ExternalInput")
    ident_d = nc.dram_tensor("ident", [128, 128], f32, kind="ExternalInput")
    dst3_d = nc.dram_tensor("dst3", [128, TOT], f32, kind="ExternalInput")
    idx_d = nc.dram_tensor("idx", [128, TOT * 16], dt.int16,
                           kind="ExternalInput")
    poolw_d = nc.dram_tensor("poolw", [NB, 128, 128], bf16,
                             kind="ExternalInput")
    pool_out = nc.dram_tensor("pool_out", [128, 128], f32,
                              kind="ExternalOutput")

    # internal DRAM ----------------------------------------------------------
    ti_d = nc.dram_tensor("ti", [cfg.NTOT, ROW], bf16)  # layer-0 table copy
    t1_d = nc.dram_tensor("t1", [cfg.NTOT, ROW], bf16)
    t2_d = nc.dram_tensor("t2", [cfg.NTOT, ROW], bf16)
    HROW = HB * 128
    # local shard halves (standalone tensors; collective ins)
    tsh_d = [[nc.dram_tensor(f"tsh{l}{h}", [HROW, ROW], bf16)
              for h in range(2)] for l in range(2)]

    def do_gather(layer, tfull, half):
        """All-gather one half of the shard table into the full table."""
        tsh = tsh_d[layer][half]
        outs = tfull.ap()[half * gpb * HROW:(half + 1) * gpb * HROW,
                          :].rearrange("(q r) c -> q r c", q=gpb)
        if timing_mode:
            for j in range(gpb):
                nc.sync.dma_start(outs[j], tsh.ap())
        else:
            nc.gpsimd.collective_compute(
                "AllGather", mybir.AluOpType.bypass, replica_groups=groups,
                ins=[tsh.ap()], outs=[outs])

    with tile.TileContext(nc) as tc:
        cpool = tc.alloc_tile_pool(name="const", bufs=1)
        rhs1 = cpool.tile([128, 3, 134], bf16, tag="rhs1")
        rhsn = cpool.tile([128, 3, 134], bf16, tag="rhsn")
        bt1 = cpool.tile([128, 134], f32, tag="bt1")
        btn = cpool.tile([128, 134], f32, tag="btn")
        iota = cpool.tile([128, 128], bf16, tag="iota")
        identf = cpool.tile([128, 128], f32, tag="identf")
        dst3 = cpool.tile([128, TOT], f32, tag="dst3")
        idx = cpool.tile([128, TOT * 16], dt.int16, tag="idx")

        nc.sync.dma_start(rhs1[:], rhs1_d.ap().rearrange("k p m -> p k m"))
        nc.sync.dma_start(rhsn[:], rhsn_d.ap().rearrange("k p m -> p k m"))
        nc.sync.dma_start(bt1[:], bt1_d.ap())
        nc.sync.dma_start(btn[:], btn_d.ap())
        nc.sync.dma_start(iota[:], iota_d.ap())
        nc.sync.dma_start(identf[:], ident_d.ap())
        nc.sync.dma_start(dst3[:], dst3_d.ap())
        nc.sync.dma_start(idx[:], idx_d.ap())

        g_pool = tc.alloc_tile_pool(name="g", bufs=4)
        w_pool = tc.alloc_tile_pool(name="w", bufs=4)
        l_pool = tc.alloc_tile_pool(name="l", bufs=18)
        psb_pool = tc.alloc_tile_pool(name="psb", bufs=2, space="PSUM")
        s_pool = tc.alloc_tile_pool(name="s", bufs=3)
        u_pool = tc.alloc_tile_pool(name="u", bufs=8)
        pst_pool = tc.alloc_tile_pool(name="pst", bufs=2, space="PSUM")
        ut_pool = tc.alloc_tile_pool(name="ut", bufs=8)
        psx_pool = tc.alloc_tile_pool(name="psx", bufs=3, space="PSUM")
        x_pool = tc.alloc_tile_pool(name="x", bufs=4)
        pw_pool = tc.alloc_tile_pool(name="pw", bufs=2)
        pp_pool = tc.alloc_tile_pool(name="pp", bufs=1, space="PSUM")

        ps_pool_acc = pp_pool.tile([128, 128], f32, tag="poolacc")

        for layer in range(3):
            tbl = (t0_d, t1_d, t2_d)[layer]
            rw = rhs1 if layer == 0 else rhsn
            bt = bt1 if layer == 0 else btn
            for b in range(NB):
                ncb = int(nc_b[b])
                c0 = int(cum[b])
                # ---- merged gather: per chunk 128 src rows + 128 dst rows
                Gt = g_pool.tile([128, 2 * NCMAX, ROW], bf16, tag="G")
                wt = w_pool.tile([128, NCMAX, 3], f32, tag="wt")
                for g0 in range(0, ncb, cfg.GCH):
                    gsz = min(cfg.GCH, ncb - g0)
                    if "gather" not in skip:
                        nc.gpsimd.dma_gather(
                            Gt[:, 2 * g0:2 * (g0 + gsz), :], tbl.ap(),
                            idx[:, 16 * (c0 + g0): 16 * (c0 + g0 + gsz)],
                            num_idxs=gsz * 256, num_idxs_reg=gsz * 256,
                            elem_size=ROW, elem_step=ROW)
                    # ---- attention weights w = exp(leaky(el_src + er_dst))
                    sl = slice(g0, g0 + gsz)
                    nc.vector.tensor_tensor(
                        wt[:, sl, :],
                        Gt[:, 2 * g0:2 * (g0 + gsz):2, cfg.EL0:cfg.EL0 + 3],
                        Gt[:, 2 * g0 + 1:2 * (g0 + gsz):2,
                           cfg.ER0:cfg.ER0 + 3], Alu.add)
                    nc.vector.scalar_tensor_tensor(
                        wt[:, sl, :], wt[:, sl, :], cfg.neg_slope,
                        wt[:, sl, :], Alu.mult, Alu.max)
                    nc.scalar.activation(wt[:, sl, :], wt[:, sl, :], Act.Exp)
                # ---- scatter-add via w-scaled one-hot matmuls
                psb = psb_pool.tile([128, 3 * RHS], f32, tag="psb")
                for c in range(ncb):
                    cc = c0 + c
                    for h in range(3):
                        lh = l_pool.tile([128, 128], bf16, tag="lh")
                        if "onehot" not in skip:
                            nc.vector.tensor_scalar(
                                lh[:], iota[:], dst3[:, cc:cc + 1],
                                wt[:, c, h:h + 1].opt(),
                                Alu.is_equal, Alu.mult)
                        if "emm" in skip:
                            continue
                        nc.tensor.matmul(
                            psb[:, RHS * h:RHS * h + RHS], lh[:],
                            Gt[:, 2 * c, 0:RHS].opt(),
                            start=(c == 0 and h == 0),
                            stop=(c == ncb - 1 and h == 2))
                # ---- epilogue: normalize, transpose, apply M|MC
                r3 = s_pool.tile([128, 3], f32, tag="r3")
                nc.vector.reciprocal(r3[:], psb[:, RHS - 1::RHS])
                psx = psx_pool.tile([128, 134], f32, tag="psx")
                for h in range(3):
                    u = u_pool.tile([128, 128], f32, tag="u")
                    nc.scalar.activation(u[:], psb[:, RHS * h:RHS * h + 128],
                                         Act.Copy, scale=r3[:, h:h + 1])
                    pst = pst_pool.tile([128, 128], f32, tag="pst")
                    nc.tensor.transpose(pst[:], u[:], identf[:])
                    uT = ut_pool.tile([128, 128], bf16, tag="uT")
                    nc.scalar.activation(uT[:], pst[:], Act.Copy)
                    nc.tensor.matmul(psx[:], uT[:], rw[:, h, :].opt(),
                                     start=(h == 0), stop=(h == 2))
                xsb = x_pool.tile([128, ROW], bf16, tag="xsb")
                nc.vector.tensor_tensor(xsb[:, 0:134], psx[:], bt[:], Alu.add)
                if layer < 2:
                    nc.vector.memset(xsb[:, cfg.ONE:ROW], 1.0)
                    half, bh = (0, b) if b < HB else (1, b - HB)
                    nc.sync.dma_start(
                        tsh_d[layer][half].ap()[bh * 128:(bh + 1) * 128, :],
                        xsb[:])
                    if b == HB - 1:
                        do_gather(layer, (t1_d, t2_d)[layer], 0)
                    elif b == NB - 1:
                        do_gather(layer, (t1_d, t2_d)[layer], 1)
                else:
                    pw = pw_pool.tile([128, 128], bf16, tag="pw")
                    nc.sync.dma_start(pw[:], poolw_d.ap()[b])
                    nc.tensor.matmul(ps_pool_acc[:], pw[:], xsb[:, 0:128],
                                     start=(b == 0), stop=(b == NB - 1))

        po = x_pool.tile([128, 128], f32, tag="po")
        nc.vector.tensor_copy(po[:], ps_pool_acc[:])
        nc.sync.dma_start(pool_out.ap(), po[:])

        for p in (pp_pool, pw_pool, x_pool, psx_pool, ut_pool, pst_pool,
                  u_pool, s_pool, psb_pool, l_pool, w_pool, g_pool, cpool):
            p.release()

    nc.compile()
    return nc


# ---------------------------------------------------------------------------
# top-level kernel
# ---------------------------------------------------------------------------

def _prepare(cfg, inputs):
    """Returns (nc_b, in_maps, host_meta)."""
    npf = np.asarray
    import ml_dtypes
    tdt = ml_dtypes.bfloat16

    per_core_edges = []
    nc_b = np.zeros(cfg.NB, np.int64)
    for br, (s, d) in enumerate((("srcA", "dstA"), ("srcB", "dstB"))):
        src = npf(inputs[s]).astype(np.int64)
        dst = npf(inputs[d]).astype(np.int64)
        for q in range(cfg.gpb):
            es, ed, cnt = _prep_edges(cfg, src, dst, q)
            per_core_edges.append((es, ed))
            nc_b = np.maximum(nc_b, -(-cnt // 128))

    iota = np.tile(np.arange(128, dtype=tdt), (128, 1))
    ident = np.eye(128, dtype=np.float32)

    in_maps = []
    host_meta = {}
    for br in range(2):
        sfx = "AB"[br]
        W1 = npf(inputs["W1" + sfx]); al1 = npf(inputs["al1" + sfx])
        ar1 = npf(inputs["ar1" + sfx]); b1 = npf(inputs["b1" + sfx])
        Wn = npf(inputs["Wn" + sfx]); aln = npf(inputs["aln" + sfx])
        arn = npf(inputs["arn" + sfx]); bn = npf(inputs["bn" + sfx])
        Wl = npf(inputs["Wl" + sfx]); bl = npf(inputs["bl" + sfx])
        gid = npf(inputs["gid" + sfx]).astype(np.int64)
        feats = npf(inputs["feats" + sfx]).astype(np.float32)
        rhs1, rhsn, bt1, btn, C1 = _branch_consts(
            cfg, W1, al1, ar1, b1, Wn, aln, arn, bn, Wl, bl)
        t0 = _table0(cfg, feats, C1, tdt)
        host_meta[sfx] = dict(gid=gid)
        for q in range(cfg.gpb):
            es, ed = per_core_edges[br * cfg.gpb + q]
            idx, d3 = _pack_core(cfg, es, ed, q, nc_b)
            poolw = np.zeros((cfg.NB, 128, 128), tdt)
            for b in range(cfg.NB):
                for i in range(min(128, cfg.SH - b * 128)):
                    n = q * cfg.SH + b * 128 + i
                    if n < cfg.N:
                        poolw[b, i, gid[n]] = 1.0
            in_maps.append({
                "t0": t0,
                "rhs1": rhs1.astype(tdt), "rhsn": rhsn.astype(tdt),
                "bt1": bt1, "btn": btn,
                "iota": iota, "ident": ident,
                "dst3": d3, "idx": idx, "poolw": poolw,
            })
    return nc_b, in_maps, host_meta


def _finalize(cfg, inputs, host_meta, pool_outs):
    """pool_outs: list of 8 [128,128] arrays -> full output [G,1] float64."""
    out = {}
    for br in range(2):
        sfx = "AB"[br]
        total = np.zeros((128, 128), np.float64)
        for q in range(cfg.gpb):
            total += pool_outs[br * cfg.gpb + q].astype(np.float64)
        gid = host_meta[sfx]["gid"]
        cnt = np.bincount(gid, minlength=128).astype(np.float64)
        out[sfx] = (total / np.maximum(cnt[:, None], 1.0))[:cfg.G]
    cat = np.concatenate([out["A"], out["B"]], axis=1)
    Wo = np.asarray(inputs["Wo"]).astype(np.float64)
    bo = np.asarray(inputs["bo"]).astype(np.float64)
    return (cat @ Wo + bo).astype(np.float64)


_CACHE = {}


def kernel(**inputs):
    cfg = Cfg(N=inputs["featsA"].shape[0], G=128)
    nc_b, in_maps, host_meta = _prepare(cfg, inputs)
    key = ("prog", tuple(nc_b.tolist()))
    if key not in _CACHE:
        _CACHE[key] = build_program(cfg, nc_b)
    nc = _CACHE[key]
    from concourse.bass_utils import run_bass_kernel_spmd
    res = run_bass_kernel_spmd(nc, in_maps, list(range(cfg.n_cores)))
    pool_outs = [r["pool_out"] for r in res.results]
    return _finalize(cfg, inputs, host_meta, pool_outs)


# revision 54
# speedup vs baseline: 1.7190x; 1.0039x over previous
"""Trainium2 Bass kernel for nn_DoubleNet (two GATNet branches + avg-pool + linear).

Strategy (8 NeuronCores), "x-gather" design:
  - Cores 0-3 run branch A, cores 4-7 run branch B (same SPMD program,
    different data). Within a branch, dst nodes are sharded across 4 cores.
  - Key algebraic refactor: for one GAT layer followed by the shared linear,
      x_next[d] = sum_h (sum_e a_e^h x[src_e]) @ M_h + blp,   M_h = W_h @ Wl_h
    so the edge phase only needs x[src] (128 cols) per edge instead of
    z[src] (384 cols), and the per-head linear maps are applied AFTER
    aggregation, per 128-dst block. el/er attention scalars are linear in x
    too (el = x @ (W_h al_h)), kept in the per-node table row.
  - Per-layer node table (DRAM, bf16 rows of 512B):
      row(n) = [x (128) | el (3) | er (3) | 1 | pad] ; row id = q*5120 + r.
    Layer-0 table is host-built from feats; later tables are written
    per-block by the epilogue and all-gathered across the branch's 4 cores
    in two halves for overlap.
  - Edge phase per dst block: ONE merged dma_gather call pulls, per 128-edge
    chunk, 128 src rows and 128 dst rows (interleaved slabs; er comes from
    the dst rows). w = exp(leaky(el+er)); per head a w-scaled one-hot matmul
    scatter-adds messages+denominator into PSUM ([x|...|1] rhs, 135 cols).
  - Epilogue per block: normalize per head (Act copy w/ scale), PE-transpose,
    3 matmuls against [M_h | M_h@Cnext] (134 cols) -> x_next and next-layer
    el/er in one shot; bias-add; row-write to the next table (layers 0,1) or
    one-hot pool matmul (layer 2). Host divides pooled sums by graph counts
    and applies the output linear.
"""

import sys

sys.path.insert(0, "/opt/trn_rl_repo")

import numpy as np


# ---------------------------------------------------------------------------
# configuration
# ---------------------------------------------------------------------------

class Cfg:
    def __init__(self, N=20000, G=128, H=3, EMB=128, F=128, n_cores=8,
                 neg_slope=0.2):
        assert F == 128 and EMB == 128 and H == 3
        self.N, self.G, self.H, self.EMB, self.F = N, G, H, EMB, F
        self.n_cores = n_cores
        self.gpb = n_cores // 2            # cores per branch
        assert N % self.gpb == 0
        self.SH = N // self.gpb            # dst nodes per core
        self.NB = -(-self.SH // 128)       # dst blocks per core
        self.SHP = self.NB * 128           # padded rows per shard
        self.NTOT = self.gpb * self.SHP    # table rows
        self.HB = self.NB // 2
        self.neg_slope = neg_slope
        # table row layout (bf16 cols): [x 0:128 | el 128:131 | er 131:134 |
        #   one @134 | pad to 256]. er window = cols 128:256 of the row.
        self.ROW = 256
        self.EL0, self.ER0, self.ONE = 128, 131, 134
        self.RHS = 135                     # matmul rhs cols [x|el|er|1]
        self.GCH = 4                       # chunks per gather call (256 idx each)
        self.WCH = 8                       # chunks per attention-weight batch
        self.dma_scratch = 32768           # SWDGE ring size driver


# ---------------------------------------------------------------------------
# host-side data prep
# ---------------------------------------------------------------------------

def _prep_edges(cfg, src, dst, q):
    """Edges of one core (dst in its shard), dst-sorted, fake rows added."""
    lo = q * cfg.SH
    sel = (dst >= lo) & (dst < lo + cfg.SH)
    es = src[sel].astype(np.int64)
    ed = (dst[sel].astype(np.int64) - lo)
    nfake = cfg.SHP - cfg.SH
    if nfake:
        es = np.concatenate([es, np.zeros(nfake, np.int64)])
        ed = np.concatenate([ed, np.arange(cfg.SH, cfg.SHP, dtype=np.int64)])
    order = np.argsort(ed, kind="stable")
    es, ed = es[order], ed[order]
    cnt = np.bincount(ed // 128, minlength=cfg.NB)
    return es, ed, cnt


def _row_of(cfg, n):
    """Global node id -> table row id.

    Rows are grouped (half, rank, local) with HROW = HB*128 rows per rank per
    half, so each all-gather half lands in one contiguous table region.
    """
    q, r = n // cfg.SH, n % cfg.SH
    HROW = cfg.HB * 128
    return np.where(r < HROW, q * HROW + r,
                    cfg.gpb * HROW + q * HROW + (r - HROW))


def _pack_core(cfg, es, ed, q, nc_b):
    """Merged (src,dst) interleaved index array + dst3 per chunk."""
    TOT = int(nc_b.sum())
    idx = np.zeros(TOT * 256, np.int16)    # per chunk: 128 src rows, 128 dst
    dst3 = np.full(TOT * 128, -1.0, np.float32)
    epos = np.searchsorted(ed, np.arange(0, cfg.SHP + 1, 128))
    c0 = 0
    for b in range(cfg.NB):
        s, e = epos[b], epos[b + 1]
        srows = _row_of(cfg, es[s:e])
        HROW = cfg.HB * 128
        dl = ed[s:e]
        drows = np.where(dl < HROW, q * HROW + dl,
                         cfg.gpb * HROW + q * HROW + (dl - HROW))
        off = ed[s:e] - b * 128
        for c in range(int(nc_b[b])):
            o, n = c * 128, min(128, (e - s) - c * 128)
            if n <= 0:
                break
            cc = c0 + c
            idx[cc * 256: cc * 256 + n] = srows[o:o + n]
            idx[cc * 256 + 128: cc * 256 + 128 + n] = drows[o:o + n]
            dst3[cc * 128: cc * 128 + n] = off[o:o + n].astype(np.float32)
        c0 += int(nc_b[b])

    def wrap(a):  # flat i -> (partition i%16, col i//16), replicated to 128
        return np.tile(a.reshape(-1, 16).T, (8, 1)).copy()

    d3 = dst3.reshape(TOT, 128).T.copy()
    return wrap(idx), d3


def _branch_consts(cfg, W1, al1, ar1, b1, Wn, aln, arn, bn, Wl, bl):
    """Per-branch folded weights: M|MC rhs tensors, bias tiles, el/er coeffs."""
    H, EMB = cfg.H, cfg.EMB

    def coeffs(W, al, ar):
        C = np.zeros((W.shape[0], 6), np.float64)
        for h in range(H):
            Wh = W[:, h * EMB:(h + 1) * EMB].astype(np.float64)
            C[:, h] = Wh @ al[h].astype(np.float64)
            C[:, 3 + h] = Wh @ ar[h].astype(np.float64)
        return C

    Wl64 = Wl.astype(np.float64)
    C1 = coeffs(W1, al1, ar1)
    Cn = coeffs(Wn, aln, arn)

    def rhsW(W):
        R = np.zeros((H, W.shape[0], 134), np.float32)
        for h in range(H):
            Wh = W[:, h * EMB:(h + 1) * EMB].astype(np.float64)
            M = Wh @ Wl64[h * EMB:(h + 1) * EMB, :]
            R[h, :, 0:128] = M.astype(np.float32)
            R[h, :, 128:134] = (M @ Cn).astype(np.float32)
        return R

    blp1 = (b1.astype(np.float64) @ Wl64 + bl.astype(np.float64))
    blpn = (bn.astype(np.float64) @ Wl64 + bl.astype(np.float64))

    def btile(blp):
        B = np.zeros((134,), np.float32)
        B[0:128] = blp.astype(np.float32)
        B[128:134] = (blp @ Cn).astype(np.float32)
        return np.tile(B[None, :], (128, 1)).copy()

    return rhsW(W1), rhsW(Wn), btile(blp1), btile(blpn), C1


def _table0(cfg, feats, C1, tdt):
    """Host-built layer-0 table [NTOT, 256] bf16 ((half, rank, local) rows)."""
    t = np.zeros((cfg.NTOT, cfg.ROW), np.float32)
    f64 = feats.astype(np.float64)
    elr = (f64 @ C1).astype(np.float32)
    rows = _row_of(cfg, np.arange(cfg.N))
    t[rows, 0:128] = feats
    t[rows, 128:134] = elr
    t[:, cfg.ONE] = 1.0
    return t.astype(tdt)


# ---------------------------------------------------------------------------
# device program
# ---------------------------------------------------------------------------

def build_program(cfg, nc_b, timing_mode=False, skip=()):
    import concourse.bass as bass
    import concourse.mybir as mybir
    import concourse.tile as tile
    from concourse import bacc

    dt = mybir.dt
    f32 = dt.float32
    bf16 = dt.bfloat16
    Alu = mybir.AluOpType
    Act = mybir.ActivationFunctionType

    NB, HB, SH, SHP = cfg.NB, cfg.HB, cfg.SH, cfg.SHP
    ROW, RHS = cfg.ROW, cfg.RHS
    TOT = int(nc_b.sum())
    NCMAX = int(nc_b.max())
    cum = np.concatenate([[0], np.cumsum(nc_b)]).astype(int)
    gpb = cfg.gpb
    groups = [list(range(gpb)), list(range(gpb, 2 * gpb))]

    nc = bacc.Bacc("TRN2", target_bir_lowering=False, debug=False,
                   num_devices=cfg.n_cores,
                   dynamic_dma_scratch_size=cfg.dma_scratch)

    # inputs -----------------------------------------------------------------
    t0_d = nc.dram_tensor("t0", [cfg.NTOT, ROW], bf16, kind="ExternalInput")
    rhs1_d = nc.dram_tensor("rhs1", [3, 128, 134], bf16, kind="ExternalInput")
    rhsn_d = nc.dram_tensor("rhsn", [3, 128, 134], bf16, kind="ExternalInput")
    bt1_d = nc.dram_tensor("bt1", [128, 134], f32, kind="ExternalInput")
    btn_d = nc.dram_tensor("btn", [128, 134], f32, kind="ExternalInput")
    iota_d = nc.dram_tensor("iota", [128, 128], bf16, kind="ExternalInput")
    ident_d = nc.dram_tensor("ident", [128, 128], f32, kind="ExternalInput")
    dst3_d = nc.dram_tensor("dst3", [128, TOT], f32, kind="ExternalInput")
    idx_d = nc.dram_tensor("idx", [128, TOT * 16], dt.int16,
                           kind="ExternalInput")
    poolw_d = nc.dram_tensor("poolw", [NB, 128, 128], bf16,
                             kind="ExternalInput")
    pool_out = nc.dram_tensor("pool_out", [128, 128], f32,
                              kind="ExternalOutput")

    # internal DRAM ----------------------------------------------------------
    ti_d = nc.dram_tensor("ti", [cfg.NTOT, ROW], bf16)  # layer-0 table copy
    t1_d = nc.dram_tensor("t1", [cfg.NTOT, ROW], bf16)
    t2_d = nc.dram_tensor("t2", [cfg.NTOT, ROW], bf16)
    HROW = HB * 128
    # local shard halves (standalone tensors; collective ins)
    tsh_d = [[nc.dram_tensor(f"tsh{l}{h}", [HROW, ROW], bf16)
              for h in range(2)] for l in range(2)]

    def do_gather(layer, tfull, half):
        """All-gather one half of the shard table into the full table."""
        tsh = tsh_d[layer][half]
        outs = tfull.ap()[half * gpb * HROW:(half + 1) * gpb * HROW,
                          :].rearrange("(q r) c -> q r c", q=gpb)
        if timing_mode:
            for j in range(gpb):
                nc.sync.dma_start(outs[j], tsh.ap())
        else:
            nc.gpsimd.collective_compute(
                "AllGather", mybir.AluOpType.bypass, replica_groups=groups,
                ins=[tsh.ap()], outs=[outs])

    with tile.TileContext(nc) as tc:
        cpool = tc.alloc_tile_pool(name="const", bufs=1)
        rhs1 = cpool.tile([128, 3, 134], bf16, tag="rhs1")
        rhsn = cpool.tile([128, 3, 134], bf16, tag="rhsn")
        bt1 = cpool.tile([128, 134], f32, tag="bt1")
        btn = cpool.tile([128, 134], f32, tag="btn")
        iota = cpool.tile([128, 128], bf16, tag="iota")
        identf = cpool.tile([128, 128], f32, tag="identf")
        dst3 = cpool.tile([128, TOT], f32, tag="dst3")
        idx = cpool.tile([128, TOT * 16], dt.int16, tag="idx")

        nc.sync.dma_start(rhs1[:], rhs1_d.ap().rearrange("k p m -> p k m"))
        nc.sync.dma_start(rhsn[:], rhsn_d.ap().rearrange("k p m -> p k m"))
        nc.sync.dma_start(bt1[:], bt1_d.ap())
        nc.sync.dma_start(btn[:], btn_d.ap())
        nc.sync.dma_start(iota[:], iota_d.ap())
        nc.sync.dma_start(identf[:], ident_d.ap())
        nc.sync.dma_start(dst3[:], dst3_d.ap())
        nc.sync.dma_start(idx[:], idx_d.ap())

        g_pool = tc.alloc_tile_pool(name="g", bufs=4)
        w_pool = tc.alloc_tile_pool(name="w", bufs=4)
        l_pool = tc.alloc_tile_pool(name="l", bufs=18)
        psb_pool = tc.alloc_tile_pool(name="psb", bufs=2, space="PSUM")
        s_pool = tc.alloc_tile_pool(name="s", bufs=3)
        u_pool = tc.alloc_tile_pool(name="u", bufs=8)
        pst_pool = tc.alloc_tile_pool(name="pst", bufs=2, space="PSUM")
        ut_pool = tc.alloc_tile_pool(name="ut", bufs=8)
        psx_pool = tc.alloc_tile_pool(name="psx", bufs=3, space="PSUM")
        x_pool = tc.alloc_tile_pool(name="x", bufs=4)
        pw_pool = tc.alloc_tile_pool(name="pw", bufs=2)
        pp_pool = tc.alloc_tile_pool(name="pp", bufs=1, space="PSUM")

        ps_pool_acc = pp_pool.tile([128, 128], f32, tag="poolacc")

        for layer in range(3):
            tbl = (t0_d, t1_d, t2_d)[layer]
            rw = rhs1 if layer == 0 else rhsn
            bt = bt1 if layer == 0 else btn
            for b in range(NB):
                ncb = int(nc_b[b])
                c0 = int(cum[b])
                # ---- merged gather: per chunk 128 src rows + 128 dst rows
                Gt = g_pool.tile([128, 2 * NCMAX, ROW], bf16, tag="G")
                wt = w_pool.tile([128, NCMAX, 3], f32, tag="wt")
                for g0 in range(0, ncb, cfg.GCH):
                    gsz = min(cfg.GCH, ncb - g0)
                    if "gather" not in skip:
                        nc.gpsimd.dma_gather(
                            Gt[:, 2 * g0:2 * (g0 + gsz), :], tbl.ap(),
                            idx[:, 16 * (c0 + g0): 16 * (c0 + g0 + gsz)],
                            num_idxs=gsz * 256, num_idxs_reg=gsz * 256,
                            elem_size=ROW, elem_step=ROW)
                for g0 in range(0, ncb, cfg.WCH):
                    gsz = min(cfg.WCH, ncb - g0)
                    # ---- attention weights w = exp(leaky(el_src + er_dst))
                    sl = slice(g0, g0 + gsz)
                    nc.vector.tensor_tensor(
                        wt[:, sl, :],
                        Gt[:, 2 * g0:2 * (g0 + gsz):2, cfg.EL0:cfg.EL0 + 3],
                        Gt[:, 2 * g0 + 1:2 * (g0 + gsz):2,
                           cfg.ER0:cfg.ER0 + 3], Alu.add)
                    nc.vector.scalar_tensor_tensor(
                        wt[:, sl, :], wt[:, sl, :], cfg.neg_slope,
                        wt[:, sl, :], Alu.mult, Alu.max)
                    nc.scalar.activation(wt[:, sl, :], wt[:, sl, :], Act.Exp)
                # ---- scatter-add via w-scaled one-hot matmuls
                psb = psb_pool.tile([128, 3 * RHS], f32, tag="psb")
                for c in range(ncb):
                    cc = c0 + c
                    lhs = []
                    for h in range(3):
                        lh = l_pool.tile([128, 128], bf16, tag="lh")
                        lhs.append(lh)
                        if "onehot" not in skip:
                            nc.vector.tensor_scalar(
                                lh[:], iota[:], dst3[:, cc:cc + 1],
                                wt[:, c, h:h + 1].opt(),
                                Alu.is_equal, Alu.mult)
                    if "emm" in skip:
                        continue
                    for h in range(3):
                        nc.tensor.matmul(
                            psb[:, RHS * h:RHS * h + RHS], lhs[h][:],
                            Gt[:, 2 * c, 0:RHS].opt(),
                            start=(c == 0 and h == 0),
                            stop=(c == ncb - 1 and h == 2))
                # ---- epilogue: normalize, transpose, apply M|MC
                r3 = s_pool.tile([128, 3], f32, tag="r3")
                nc.vector.reciprocal(r3[:], psb[:, RHS - 1::RHS])
                psx = psx_pool.tile([128, 134], f32, tag="psx")
                for h in range(3):
                    u = u_pool.tile([128, 128], f32, tag="u")
                    nc.scalar.activation(u[:], psb[:, RHS * h:RHS * h + 128],
                                         Act.Copy, scale=r3[:, h:h + 1])
                    pst = pst_pool.tile([128, 128], f32, tag="pst")
                    nc.tensor.transpose(pst[:], u[:], identf[:])
                    uT = ut_pool.tile([128, 128], bf16, tag="uT")
                    nc.scalar.activation(uT[:], pst[:], Act.Copy)
                    nc.tensor.matmul(psx[:], uT[:], rw[:, h, :].opt(),
                                     start=(h == 0), stop=(h == 2))
                xsb = x_pool.tile([128, ROW], bf16, tag="xsb")
                nc.vector.tensor_tensor(xsb[:, 0:134], psx[:], bt[:], Alu.add)
                if layer < 2:
                    nc.vector.memset(xsb[:, cfg.ONE:ROW], 1.0)
                    half, bh = (0, b) if b < HB else (1, b - HB)
                    nc.sync.dma_start(
                        tsh_d[layer][half].ap()[bh * 128:(bh + 1) * 128, :],
                        xsb[:])
                    if b == HB - 1:
                        do_gather(layer, (t1_d, t2_d)[layer], 0)
                    elif b == NB - 1:
                        do_gather(layer, (t1_d, t2_d)[layer], 1)
                else:
                    pw = pw_pool.tile([128, 128], bf16, tag="pw")
                    nc.sync.dma_start(pw[:], poolw_d.ap()[b])
                    nc.tensor.matmul(ps_pool_acc[:], pw[:], xsb[:, 0:128],
                                     start=(b == 0), stop=(b == NB - 1))

        po = x_pool.tile([128, 128], f32, tag="po")
        nc.vector.tensor_copy(po[:], ps_pool_acc[:])
        nc.sync.dma_start(pool_out.ap(), po[:])

        for p in (pp_pool, pw_pool, x_pool, psx_pool, ut_pool, pst_pool,
                  u_pool, s_pool, psb_pool, l_pool, w_pool, g_pool, cpool):
            p.release()

    nc.compile()
    return nc


# ---------------------------------------------------------------------------
# top-level kernel
# ---------------------------------------------------------------------------

def _prepare(cfg, inputs):
    """Returns (nc_b, in_maps, host_meta)."""
    npf = np.asarray
    import ml_dtypes
    tdt = ml_dtypes.bfloat16

    per_core_edges = []
    nc_b = np.zeros(cfg.NB, np.int64)
    for br, (s, d) in enumerate((("srcA", "dstA"), ("srcB", "dstB"))):
        src = npf(inputs[s]).astype(np.int64)
        dst = npf(inputs[d]).astype(np.int64)
        for q in range(cfg.gpb):
            es, ed, cnt = _prep_edges(cfg, src, dst, q)
            per_core_edges.append((es, ed))
            nc_b = np.maximum(nc_b, -(-cnt // 128))

    iota = np.tile(np.arange(128, dtype=tdt), (128, 1))
    ident = np.eye(128, dtype=np.float32)

    in_maps = []
    host_meta = {}
    for br in range(2):
        sfx = "AB"[br]
        W1 = npf(inputs["W1" + sfx]); al1 = npf(inputs["al1" + sfx])
        ar1 = npf(inputs["ar1" + sfx]); b1 = npf(inputs["b1" + sfx])
        Wn = npf(inputs["Wn" + sfx]); aln = npf(inputs["aln" + sfx])
        arn = npf(inputs["arn" + sfx]); bn = npf(inputs["bn" + sfx])
        Wl = npf(inputs["Wl" + sfx]); bl = npf(inputs["bl" + sfx])
        gid = npf(inputs["gid" + sfx]).astype(np.int64)
        feats = npf(inputs["feats" + sfx]).astype(np.float32)
        rhs1, rhsn, bt1, btn, C1 = _branch_consts(
            cfg, W1, al1, ar1, b1, Wn, aln, arn, bn, Wl, bl)
        t0 = _table0(cfg, feats, C1, tdt)
        host_meta[sfx] = dict(gid=gid)
        for q in range(cfg.gpb):
            es, ed = per_core_edges[br * cfg.gpb + q]
            idx, d3 = _pack_core(cfg, es, ed, q, nc_b)
            poolw = np.zeros((cfg.NB, 128, 128), tdt)
            for b in range(cfg.NB):
                for i in range(min(128, cfg.SH - b * 128)):
                    n = q * cfg.SH + b * 128 + i
                    if n < cfg.N:
                        poolw[b, i, gid[n]] = 1.0
            in_maps.append({
                "t0": t0,
                "rhs1": rhs1.astype(tdt), "rhsn": rhsn.astype(tdt),
                "bt1": bt1, "btn": btn,
                "iota": iota, "ident": ident,
                "dst3": d3, "idx": idx, "poolw": poolw,
            })
    return nc_b, in_maps, host_meta


def _finalize(cfg, inputs, host_meta, pool_outs):
    """pool_outs: list of 8 [128,128] arrays -> full output [G,1] float64."""
    out = {}
    for br in range(2):
        sfx = "AB"[br]
        total = np.zeros((128, 128), np.float64)
        for q in range(cfg.gpb):
            total += pool_outs[br * cfg.gpb + q].astype(np.float64)
        gid = host_meta[sfx]["gid"]
        cnt = np.bincount(gid, minlength=128).astype(np.float64)
        out[sfx] = (total / np.maximum(cnt[:, None], 1.0))[:cfg.G]
    cat = np.concatenate([out["A"], out["B"]], axis=1)
    Wo = np.asarray(inputs["Wo"]).astype(np.float64)
    bo = np.asarray(inputs["bo"]).astype(np.float64)
    return (cat @ Wo + bo).astype(np.float64)


_CACHE = {}


def kernel(**inputs):
    cfg = Cfg(N=inputs["featsA"].shape[0], G=128)
    nc_b, in_maps, host_meta = _prepare(cfg, inputs)
    key = ("prog", tuple(nc_b.tolist()))
    if key not in _CACHE:
        _CACHE[key] = build_program(cfg, nc_b)
    nc = _CACHE[key]
    from concourse.bass_utils import run_bass_kernel_spmd
    res = run_bass_kernel_spmd(nc, in_maps, list(range(cfg.n_cores)))
    pool_outs = [r["pool_out"] for r in res.results]
    return _finalize(cfg, inputs, host_meta, pool_outs)


# revision 55
# speedup vs baseline: 1.7216x; 1.0015x over previous
"""Trainium2 Bass kernel for nn_DoubleNet (two GATNet branches + avg-pool + linear).

Strategy (8 NeuronCores), "x-gather" design:
  - Cores 0-3 run branch A, cores 4-7 run branch B (same SPMD program,
    different data). Within a branch, dst nodes are sharded across 4 cores.
  - Key algebraic refactor: for one GAT layer followed by the shared linear,
      x_next[d] = sum_h (sum_e a_e^h x[src_e]) @ M_h + blp,   M_h = W_h @ Wl_h
    so the edge phase only needs x[src] (128 cols) per edge instead of
    z[src] (384 cols), and the per-head linear maps are applied AFTER
    aggregation, per 128-dst block. el/er attention scalars are linear in x
    too (el = x @ (W_h al_h)), kept in the per-node table row.
  - Per-layer node table (DRAM, bf16 rows of 512B):
      row(n) = [x (128) | el (3) | er (3) | 1 | pad] ; row id = q*5120 + r.
    Layer-0 table is host-built from feats; later tables are written
    per-block by the epilogue and all-gathered across the branch's 4 cores
    in two halves for overlap.
  - Edge phase per dst block: ONE merged dma_gather call pulls, per 128-edge
    chunk, 128 src rows and 128 dst rows (interleaved slabs; er comes from
    the dst rows). w = exp(leaky(el+er)); per head a w-scaled one-hot matmul
    scatter-adds messages+denominator into PSUM ([x|...|1] rhs, 135 cols).
  - Epilogue per block: normalize per head (Act copy w/ scale), PE-transpose,
    3 matmuls against [M_h | M_h@Cnext] (134 cols) -> x_next and next-layer
    el/er in one shot; bias-add; row-write to the next table (layers 0,1) or
    one-hot pool matmul (layer 2). Host divides pooled sums by graph counts
    and applies the output linear.
"""

import sys

sys.path.insert(0, "/opt/trn_rl_repo")

import numpy as np


# ---------------------------------------------------------------------------
# configuration
# ---------------------------------------------------------------------------

class Cfg:
    def __init__(self, N=20000, G=128, H=3, EMB=128, F=128, n_cores=8,
                 neg_slope=0.2):
        assert F == 128 and EMB == 128 and H == 3
        self.N, self.G, self.H, self.EMB, self.F = N, G, H, EMB, F
        self.n_cores = n_cores
        self.gpb = n_cores // 2            # cores per branch
        assert N % self.gpb == 0
        self.SH = N // self.gpb            # dst nodes per core
        self.NB = -(-self.SH // 128)       # dst blocks per core
        self.SHP = self.NB * 128           # padded rows per shard
        self.NTOT = self.gpb * self.SHP    # table rows
        self.HB = self.NB // 2
        self.neg_slope = neg_slope
        # table row layout (bf16 cols): [x 0:128 | el 128:131 | er 131:134 |
        #   one @134 | pad to 256]. er window = cols 128:256 of the row.
        self.ROW = 256
        self.EL0, self.ER0, self.ONE = 128, 131, 134
        self.RHS = 135                     # matmul rhs cols [x|el|er|1]
        self.GCH = 4                       # chunks per gather call (256 idx each)
        self.WCH = 8                       # chunks per attention-weight batch
        self.dma_scratch = 32768           # SWDGE ring size driver


# ---------------------------------------------------------------------------
# host-side data prep
# ---------------------------------------------------------------------------

def _prep_edges(cfg, src, dst, q):
    """Edges of one core (dst in its shard), dst-sorted, fake rows added."""
    lo = q * cfg.SH
    sel = (dst >= lo) & (dst < lo + cfg.SH)
    es = src[sel].astype(np.int64)
    ed = (dst[sel].astype(np.int64) - lo)
    nfake = cfg.SHP - cfg.SH
    if nfake:
        es = np.concatenate([es, np.zeros(nfake, np.int64)])
        ed = np.concatenate([ed, np.arange(cfg.SH, cfg.SHP, dtype=np.int64)])
    order = np.argsort(ed, kind="stable")
    es, ed = es[order], ed[order]
    cnt = np.bincount(ed // 128, minlength=cfg.NB)
    return es, ed, cnt


def _row_of(cfg, n):
    """Global node id -> table row id.

    Rows are grouped (half, rank, local) with HROW = HB*128 rows per rank per
    half, so each all-gather half lands in one contiguous table region.
    """
    q, r = n // cfg.SH, n % cfg.SH
    HROW = cfg.HB * 128
    return np.where(r < HROW, q * HROW + r,
                    cfg.gpb * HROW + q * HROW + (r - HROW))


def _pack_core(cfg, es, ed, q, nc_b):
    """Merged (src,dst) interleaved index array + dst3 per chunk."""
    TOT = int(nc_b.sum())
    idx = np.zeros(TOT * 256, np.int16)    # per chunk: 128 src rows, 128 dst
    dst3 = np.full(TOT * 128, -1.0, np.float32)
    epos = np.searchsorted(ed, np.arange(0, cfg.SHP + 1, 128))
    c0 = 0
    for b in range(cfg.NB):
        s, e = epos[b], epos[b + 1]
        srows = _row_of(cfg, es[s:e])
        HROW = cfg.HB * 128
        dl = ed[s:e]
        drows = np.where(dl < HROW, q * HROW + dl,
                         cfg.gpb * HROW + q * HROW + (dl - HROW))
        off = ed[s:e] - b * 128
        for c in range(int(nc_b[b])):
            o, n = c * 128, min(128, (e - s) - c * 128)
            if n <= 0:
                break
            cc = c0 + c
            idx[cc * 256: cc * 256 + n] = srows[o:o + n]
            idx[cc * 256 + 128: cc * 256 + 128 + n] = drows[o:o + n]
            dst3[cc * 128: cc * 128 + n] = off[o:o + n].astype(np.float32)
        c0 += int(nc_b[b])

    def wrap(a):  # flat i -> (partition i%16, col i//16), replicated to 128
        return np.tile(a.reshape(-1, 16).T, (8, 1)).copy()

    d3 = dst3.reshape(TOT, 128).T.copy()
    return wrap(idx), d3


def _branch_consts(cfg, W1, al1, ar1, b1, Wn, aln, arn, bn, Wl, bl):
    """Per-branch folded weights: M|MC rhs tensors, bias tiles, el/er coeffs."""
    H, EMB = cfg.H, cfg.EMB

    def coeffs(W, al, ar):
        C = np.zeros((W.shape[0], 6), np.float64)
        for h in range(H):
            Wh = W[:, h * EMB:(h + 1) * EMB].astype(np.float64)
            C[:, h] = Wh @ al[h].astype(np.float64)
            C[:, 3 + h] = Wh @ ar[h].astype(np.float64)
        return C

    Wl64 = Wl.astype(np.float64)
    C1 = coeffs(W1, al1, ar1)
    Cn = coeffs(Wn, aln, arn)

    def rhsW(W):
        R = np.zeros((H, W.shape[0], 134), np.float32)
        for h in range(H):
            Wh = W[:, h * EMB:(h + 1) * EMB].astype(np.float64)
            M = Wh @ Wl64[h * EMB:(h + 1) * EMB, :]
            R[h, :, 0:128] = M.astype(np.float32)
            R[h, :, 128:134] = (M @ Cn).astype(np.float32)
        return R

    blp1 = (b1.astype(np.float64) @ Wl64 + bl.astype(np.float64))
    blpn = (bn.astype(np.float64) @ Wl64 + bl.astype(np.float64))

    def btile(blp):
        B = np.zeros((134,), np.float32)
        B[0:128] = blp.astype(np.float32)
        B[128:134] = (blp @ Cn).astype(np.float32)
        return np.tile(B[None, :], (128, 1)).copy()

    return rhsW(W1), rhsW(Wn), btile(blp1), btile(blpn), C1


def _table0(cfg, feats, C1, tdt):
    """Host-built layer-0 table [NTOT, 256] bf16 ((half, rank, local) rows)."""
    t = np.zeros((cfg.NTOT, cfg.ROW), np.float32)
    f64 = feats.astype(np.float64)
    elr = (f64 @ C1).astype(np.float32)
    rows = _row_of(cfg, np.arange(cfg.N))
    t[rows, 0:128] = feats
    t[rows, 128:134] = elr
    t[:, cfg.ONE] = 1.0
    return t.astype(tdt)


# ---------------------------------------------------------------------------
# device program
# ---------------------------------------------------------------------------

def build_program(cfg, nc_b, timing_mode=False, skip=()):
    import concourse.bass as bass
    import concourse.mybir as mybir
    import concourse.tile as tile
    from concourse import bacc

    dt = mybir.dt
    f32 = dt.float32
    bf16 = dt.bfloat16
    Alu = mybir.AluOpType
    Act = mybir.ActivationFunctionType

    NB, HB, SH, SHP = cfg.NB, cfg.HB, cfg.SH, cfg.SHP
    ROW, RHS = cfg.ROW, cfg.RHS
    TOT = int(nc_b.sum())
    NCMAX = int(nc_b.max())
    cum = np.concatenate([[0], np.cumsum(nc_b)]).astype(int)
    gpb = cfg.gpb
    groups = [list(range(gpb)), list(range(gpb, 2 * gpb))]

    nc = bacc.Bacc("TRN2", target_bir_lowering=False, debug=False,
                   num_devices=cfg.n_cores,
                   dynamic_dma_scratch_size=cfg.dma_scratch)

    # inputs -----------------------------------------------------------------
    t0_d = nc.dram_tensor("t0", [cfg.NTOT, ROW], bf16, kind="ExternalInput")
    rhs1_d = nc.dram_tensor("rhs1", [3, 128, 134], bf16, kind="ExternalInput")
    rhsn_d = nc.dram_tensor("rhsn", [3, 128, 134], bf16, kind="ExternalInput")
    bt1_d = nc.dram_tensor("bt1", [128, 134], f32, kind="ExternalInput")
    btn_d = nc.dram_tensor("btn", [128, 134], f32, kind="ExternalInput")
    iota_d = nc.dram_tensor("iota", [128, 128], bf16, kind="ExternalInput")
    ident_d = nc.dram_tensor("ident", [128, 128], f32, kind="ExternalInput")
    dst3_d = nc.dram_tensor("dst3", [128, TOT], f32, kind="ExternalInput")
    idx_d = nc.dram_tensor("idx", [128, TOT * 16], dt.int16,
                           kind="ExternalInput")
    poolw_d = nc.dram_tensor("poolw", [NB, 128, 128], bf16,
                             kind="ExternalInput")
    pool_out = nc.dram_tensor("pool_out", [128, 128], f32,
                              kind="ExternalOutput")

    # internal DRAM ----------------------------------------------------------
    ti_d = nc.dram_tensor("ti", [cfg.NTOT, ROW], bf16)  # layer-0 table copy
    t1_d = nc.dram_tensor("t1", [cfg.NTOT, ROW], bf16)
    t2_d = nc.dram_tensor("t2", [cfg.NTOT, ROW], bf16)
    HROW = HB * 128
    # local shard halves (standalone tensors; collective ins)
    tsh_d = [[nc.dram_tensor(f"tsh{l}{h}", [HROW, ROW], bf16)
              for h in range(2)] for l in range(2)]

    def do_gather(layer, tfull, half):
        """All-gather one half of the shard table into the full table."""
        tsh = tsh_d[layer][half]
        outs = tfull.ap()[half * gpb * HROW:(half + 1) * gpb * HROW,
                          :].rearrange("(q r) c -> q r c", q=gpb)
        if timing_mode:
            for j in range(gpb):
                nc.sync.dma_start(outs[j], tsh.ap())
        else:
            nc.gpsimd.collective_compute(
                "AllGather", mybir.AluOpType.bypass, replica_groups=groups,
                ins=[tsh.ap()], outs=[outs])

    with tile.TileContext(nc) as tc:
        cpool = tc.alloc_tile_pool(name="const", bufs=1)
        rhs1 = cpool.tile([128, 3, 134], bf16, tag="rhs1")
        rhsn = cpool.tile([128, 3, 134], bf16, tag="rhsn")
        bt1 = cpool.tile([128, 134], f32, tag="bt1")
        btn = cpool.tile([128, 134], f32, tag="btn")
        iota = cpool.tile([128, 128], bf16, tag="iota")
        identf = cpool.tile([128, 128], f32, tag="identf")
        dst3 = cpool.tile([128, TOT], f32, tag="dst3")
        idx = cpool.tile([128, TOT * 16], dt.int16, tag="idx")

        nc.sync.dma_start(rhs1[:], rhs1_d.ap().rearrange("k p m -> p k m"))
        nc.sync.dma_start(rhsn[:], rhsn_d.ap().rearrange("k p m -> p k m"))
        nc.sync.dma_start(bt1[:], bt1_d.ap())
        nc.sync.dma_start(btn[:], btn_d.ap())
        nc.sync.dma_start(iota[:], iota_d.ap())
        nc.sync.dma_start(identf[:], ident_d.ap())
        nc.sync.dma_start(dst3[:], dst3_d.ap())
        # split the idx load so block 0's indices (and the first gather)
        # don't wait for the whole 2.9MB constant transfer
        isp = 16 * int(nc_b[0])
        nc.sync.dma_start(idx[:, 0:isp], idx_d.ap()[:, 0:isp])
        nc.sync.dma_start(idx[:, isp:], idx_d.ap()[:, isp:])

        g_pool = tc.alloc_tile_pool(name="g", bufs=4)
        w_pool = tc.alloc_tile_pool(name="w", bufs=4)
        l_pool = tc.alloc_tile_pool(name="l", bufs=18)
        psb_pool = tc.alloc_tile_pool(name="psb", bufs=2, space="PSUM")
        s_pool = tc.alloc_tile_pool(name="s", bufs=3)
        u_pool = tc.alloc_tile_pool(name="u", bufs=8)
        pst_pool = tc.alloc_tile_pool(name="pst", bufs=2, space="PSUM")
        ut_pool = tc.alloc_tile_pool(name="ut", bufs=8)
        psx_pool = tc.alloc_tile_pool(name="psx", bufs=3, space="PSUM")
        x_pool = tc.alloc_tile_pool(name="x", bufs=4)
        pw_pool = tc.alloc_tile_pool(name="pw", bufs=2)
        pp_pool = tc.alloc_tile_pool(name="pp", bufs=1, space="PSUM")

        ps_pool_acc = pp_pool.tile([128, 128], f32, tag="poolacc")

        for layer in range(3):
            tbl = (t0_d, t1_d, t2_d)[layer]
            rw = rhs1 if layer == 0 else rhsn
            bt = bt1 if layer == 0 else btn
            for b in range(NB):
                ncb = int(nc_b[b])
                c0 = int(cum[b])
                # ---- merged gather: per chunk 128 src rows + 128 dst rows
                Gt = g_pool.tile([128, 2 * NCMAX, ROW], bf16, tag="G")
                wt = w_pool.tile([128, NCMAX, 3], f32, tag="wt")
                for g0 in range(0, ncb, cfg.GCH):
                    gsz = min(cfg.GCH, ncb - g0)
                    if "gather" not in skip:
                        nc.gpsimd.dma_gather(
                            Gt[:, 2 * g0:2 * (g0 + gsz), :], tbl.ap(),
                            idx[:, 16 * (c0 + g0): 16 * (c0 + g0 + gsz)],
                            num_idxs=gsz * 256, num_idxs_reg=gsz * 256,
                            elem_size=ROW, elem_step=ROW)
                for g0 in range(0, ncb, cfg.WCH):
                    gsz = min(cfg.WCH, ncb - g0)
                    # ---- attention weights w = exp(leaky(el_src + er_dst))
                    sl = slice(g0, g0 + gsz)
                    nc.vector.tensor_tensor(
                        wt[:, sl, :],
                        Gt[:, 2 * g0:2 * (g0 + gsz):2, cfg.EL0:cfg.EL0 + 3],
                        Gt[:, 2 * g0 + 1:2 * (g0 + gsz):2,
                           cfg.ER0:cfg.ER0 + 3], Alu.add)
                    nc.vector.scalar_tensor_tensor(
                        wt[:, sl, :], wt[:, sl, :], cfg.neg_slope,
                        wt[:, sl, :], Alu.mult, Alu.max)
                    nc.scalar.activation(wt[:, sl, :], wt[:, sl, :], Act.Exp)
                # ---- scatter-add via w-scaled one-hot matmuls
                psb = psb_pool.tile([128, 3 * RHS], f32, tag="psb")
                for c in range(ncb):
                    cc = c0 + c
                    lhs = []
                    for h in range(3):
                        lh = l_pool.tile([128, 128], bf16, tag="lh")
                        lhs.append(lh)
                        if "onehot" not in skip:
                            nc.vector.tensor_scalar(
                                lh[:], iota[:], dst3[:, cc:cc + 1],
                                wt[:, c, h:h + 1].opt(),
                                Alu.is_equal, Alu.mult)
                    if "emm" in skip:
                        continue
                    for h in range(3):
                        nc.tensor.matmul(
                            psb[:, RHS * h:RHS * h + RHS], lhs[h][:],
                            Gt[:, 2 * c, 0:RHS].opt(),
                            start=(c == 0 and h == 0),
                            stop=(c == ncb - 1 and h == 2))
                # ---- epilogue: normalize, transpose, apply M|MC
                r3 = s_pool.tile([128, 3], f32, tag="r3")
                nc.vector.reciprocal(r3[:], psb[:, RHS - 1::RHS])
                psx = psx_pool.tile([128, 134], f32, tag="psx")
                for h in range(3):
                    u = u_pool.tile([128, 128], f32, tag="u")
                    nc.scalar.activation(u[:], psb[:, RHS * h:RHS * h + 128],
                                         Act.Copy, scale=r3[:, h:h + 1])
                    pst = pst_pool.tile([128, 128], f32, tag="pst")
                    nc.tensor.transpose(pst[:], u[:], identf[:])
                    uT = ut_pool.tile([128, 128], bf16, tag="uT")
                    nc.scalar.activation(uT[:], pst[:], Act.Copy)
                    nc.tensor.matmul(psx[:], uT[:], rw[:, h, :].opt(),
                                     start=(h == 0), stop=(h == 2))
                xsb = x_pool.tile([128, ROW], bf16, tag="xsb")
                nc.vector.tensor_tensor(xsb[:, 0:134], psx[:], bt[:], Alu.add)
                if layer < 2:
                    nc.vector.memset(xsb[:, cfg.ONE:ROW], 1.0)
                    half, bh = (0, b) if b < HB else (1, b - HB)
                    nc.sync.dma_start(
                        tsh_d[layer][half].ap()[bh * 128:(bh + 1) * 128, :],
                        xsb[:])
                    if b == HB - 1:
                        do_gather(layer, (t1_d, t2_d)[layer], 0)
                    elif b == NB - 1:
                        do_gather(layer, (t1_d, t2_d)[layer], 1)
                else:
                    pw = pw_pool.tile([128, 128], bf16, tag="pw")
                    nc.sync.dma_start(pw[:], poolw_d.ap()[b])
                    nc.tensor.matmul(ps_pool_acc[:], pw[:], xsb[:, 0:128],
                                     start=(b == 0), stop=(b == NB - 1))

        po = x_pool.tile([128, 128], f32, tag="po")
        nc.vector.tensor_copy(po[:], ps_pool_acc[:])
        nc.sync.dma_start(pool_out.ap(), po[:])

        for p in (pp_pool, pw_pool, x_pool, psx_pool, ut_pool, pst_pool,
                  u_pool, s_pool, psb_pool, l_pool, w_pool, g_pool, cpool):
            p.release()

    nc.compile()
    return nc


# ---------------------------------------------------------------------------
# top-level kernel
# ---------------------------------------------------------------------------

def _prepare(cfg, inputs):
    """Returns (nc_b, in_maps, host_meta)."""
    npf = np.asarray
    import ml_dtypes
    tdt = ml_dtypes.bfloat16

    per_core_edges = []
    nc_b = np.zeros(cfg.NB, np.int64)
    for br, (s, d) in enumerate((("srcA", "dstA"), ("srcB", "dstB"))):
        src = npf(inputs[s]).astype(np.int64)
        dst = npf(inputs[d]).astype(np.int64)
        for q in range(cfg.gpb):
            es, ed, cnt = _prep_edges(cfg, src, dst, q)
            per_core_edges.append((es, ed))
            nc_b = np.maximum(nc_b, -(-cnt // 128))

    iota = np.tile(np.arange(128, dtype=tdt), (128, 1))
    ident = np.eye(128, dtype=np.float32)

    in_maps = []
    host_meta = {}
    for br in range(2):
        sfx = "AB"[br]
        W1 = npf(inputs["W1" + sfx]); al1 = npf(inputs["al1" + sfx])
        ar1 = npf(inputs["ar1" + sfx]); b1 = npf(inputs["b1" + sfx])
        Wn = npf(inputs["Wn" + sfx]); aln = npf(inputs["aln" + sfx])
        arn = npf(inputs["arn" + sfx]); bn = npf(inputs["bn" + sfx])
        Wl = npf(inputs["Wl" + sfx]); bl = npf(inputs["bl" + sfx])
        gid = npf(inputs["gid" + sfx]).astype(np.int64)
        feats = npf(inputs["feats" + sfx]).astype(np.float32)
        rhs1, rhsn, bt1, btn, C1 = _branch_consts(
            cfg, W1, al1, ar1, b1, Wn, aln, arn, bn, Wl, bl)
        t0 = _table0(cfg, feats, C1, tdt)
        host_meta[sfx] = dict(gid=gid)
        for q in range(cfg.gpb):
            es, ed = per_core_edges[br * cfg.gpb + q]
            idx, d3 = _pack_core(cfg, es, ed, q, nc_b)
            poolw = np.zeros((cfg.NB, 128, 128), tdt)
            for b in range(cfg.NB):
                for i in range(min(128, cfg.SH - b * 128)):
                    n = q * cfg.SH + b * 128 + i
                    if n < cfg.N:
                        poolw[b, i, gid[n]] = 1.0
            in_maps.append({
                "t0": t0,
                "rhs1": rhs1.astype(tdt), "rhsn": rhsn.astype(tdt),
                "bt1": bt1, "btn": btn,
                "iota": iota, "ident": ident,
                "dst3": d3, "idx": idx, "poolw": poolw,
            })
    return nc_b, in_maps, host_meta


def _finalize(cfg, inputs, host_meta, pool_outs):
    """pool_outs: list of 8 [128,128] arrays -> full output [G,1] float64."""
    out = {}
    for br in range(2):
        sfx = "AB"[br]
        total = np.zeros((128, 128), np.float64)
        for q in range(cfg.gpb):
            total += pool_outs[br * cfg.gpb + q].astype(np.float64)
        gid = host_meta[sfx]["gid"]
        cnt = np.bincount(gid, minlength=128).astype(np.float64)
        out[sfx] = (total / np.maximum(cnt[:, None], 1.0))[:cfg.G]
    cat = np.concatenate([out["A"], out["B"]], axis=1)
    Wo = np.asarray(inputs["Wo"]).astype(np.float64)
    bo = np.asarray(inputs["bo"]).astype(np.float64)
    return (cat @ Wo + bo).astype(np.float64)


_CACHE = {}


def kernel(**inputs):
    cfg = Cfg(N=inputs["featsA"].shape[0], G=128)
    nc_b, in_maps, host_meta = _prepare(cfg, inputs)
    key = ("prog", tuple(nc_b.tolist()))
    if key not in _CACHE:
        _CACHE[key] = build_program(cfg, nc_b)
    nc = _CACHE[key]
    from concourse.bass_utils import run_bass_kernel_spmd
    res = run_bass_kernel_spmd(nc, in_maps, list(range(cfg.n_cores)))
    pool_outs = [r["pool_out"] for r in res.results]
    return _finalize(cfg, inputs, host_meta, pool_outs)
